# revision 1
# baseline (speedup 1.0000x reference)
"""GQA kernel for Trainium2, 8 NeuronCores.

Problem: B=2, T=2048, D=2048, 16 query heads / 2 KV heads, d_head=128, causal.

Sharding: core c -> batch b = c//4, head-quarter q = c%4 (query heads
4q..4q+3, kv head q//2). Each core computes its 4 heads' attention and a
partial output projection (its Wo rows); host sums the 4 partials per batch
and adds bo.

Host marshalling: weights and x are pre-cast to bf16 (same rounding the
kernel would do on-chip) and x is supplied transposed (xT = x[b].T), which
is the layout every projection matmul consumes.

On-core dataflow (bf16 matmuls, fp32 PSUM accum), interleaved in 4 rounds
over 512-wide t-slices so PE stays continuously fed:
  round j: project KT/QT/VT for slice j; PE-transpose VT -> V natural;
           attention (h, j) for all 4 heads over tk blocks 0..4j+3
           (S_T tiles [tk,tq]; exp on ACT; causal zeroing of the diagonal
           blocks on GpSimd post-exp; OT accum on PE; row-sum accum on DVE
           with one fp32 ones-matmul per (h,j) for the partition reduction);
           output projection for the 4 t-tiles of slice j.
Engine budget: PE ~matmuls only, ACT ~exp+proj epilogues, DVE ~copies+
row-sum+normalize, GpSimd ~causal masks, 4 DMA queues for input streaming.
"""

import numpy as np
import ml_dtypes
from contextlib import ExitStack

import concourse.bass as bass
from concourse import bacc
import concourse.mybir as mybir
import concourse.tile as tile
from concourse.bass_utils import run_bass_kernel_spmd
from concourse.masks import make_identity

F32 = mybir.dt.float32
BF16 = mybir.dt.bfloat16

D = 2048
T = 2048
DH = 128
B = 2
HPC = 4            # query heads per core
NCORES = 8
SCALE = 1.0 / float(np.sqrt(128.0))

_CACHE = {}


def _build_nc():
    nc = bacc.Bacc("TRN2", target_bir_lowering=False, debug=False,
                   num_devices=NCORES)

    xt = nc.dram_tensor("xt", [D, T], BF16, kind="ExternalInput")
    wq = nc.dram_tensor("wq", [D, HPC * DH], BF16, kind="ExternalInput")
    wk = nc.dram_tensor("wk", [D, DH], BF16, kind="ExternalInput")
    wv = nc.dram_tensor("wv", [D, DH], BF16, kind="ExternalInput")
    wo = nc.dram_tensor("wo", [HPC * DH, D], BF16, kind="ExternalInput")
    bqm = nc.dram_tensor("bqm", [DH, HPC], F32, kind="ExternalInput")
    bkm = nc.dram_tensor("bkm", [DH, 1], F32, kind="ExternalInput")
    bvm = nc.dram_tensor("bvm", [DH, 1], F32, kind="ExternalInput")
    part = nc.dram_tensor("part", [T, D], F32, kind="ExternalOutput")

    with ExitStack() as ctx:
        tc = ctx.enter_context(tile.TileContext(nc))
        persist = ctx.enter_context(tc.tile_pool(name="persist", bufs=1))
        work = ctx.enter_context(tc.tile_pool(name="work", bufs=3))
        psum = ctx.enter_context(tc.tile_pool(name="psum", bufs=2, space="PSUM"))

        # ---- constants ----
        ones32 = persist.tile([128, 128], F32, tag="ones32", name="ones32")
        nc.vector.memset(ones32, 1.0)
        ident = persist.tile([128, 128], BF16, tag="ident", name="ident")
        make_identity(nc, ident)

        bq_sb = persist.tile([DH, HPC], F32, tag="bq", name="bq_sb")
        nc.sync.dma_start(out=bq_sb, in_=bqm[:, :])
        bk_sb = persist.tile([DH, 1], F32, tag="bk", name="bk_sb")
        nc.sync.dma_start(out=bk_sb, in_=bkm[:, :])
        bv_sb = persist.tile([DH, 1], F32, tag="bv", name="bv_sb")
        nc.sync.dma_start(out=bv_sb, in_=bvm[:, :])

        # ---- inputs -> SBUF (already bf16), streamed on 4 DMA queues in
        # consumption order: wk, xT slice 0, wq, wv, xT slices 1-3, wo ----
        queues = [nc.sync, nc.scalar, nc.gpsimd]
        _qi = [0]

        def dma(out, in_):
            queues[_qi[0] % 3].dma_start(out=out, in_=in_)
            _qi[0] += 1

        xT = [persist.tile([128, T], BF16, tag=f"xT{kb}", name=f"xT{kb}")
              for kb in range(16)]
        wq_sb = []
        wk_sb = []
        wv_sb = []
        for kb in range(16):
            wkt = persist.tile([128, 128], BF16, tag=f"wk{kb}", name=f"wk_sb{kb}")
            dma(wkt, wk[kb * 128:(kb + 1) * 128, :])
            wk_sb.append(wkt)
        for kb in range(16):
            dma(xT[kb][:, 0:512], xt[kb * 128:(kb + 1) * 128, 0:512])
        for kb in range(16):
            wqt = persist.tile([128, 512], BF16, tag=f"wq{kb}", name=f"wq_sb{kb}")
            dma(wqt, wq[kb * 128:(kb + 1) * 128, :])
            wq_sb.append(wqt)
        for kb in range(16):
            wvt = persist.tile([128, 128], BF16, tag=f"wv{kb}", name=f"wv_sb{kb}")
            dma(wvt, wv[kb * 128:(kb + 1) * 128, :])
            wv_sb.append(wvt)
        for js in range(1, 4):
            for kb in range(16):
                dma(xT[kb][:, js * 512:(js + 1) * 512],
                    xt[kb * 128:(kb + 1) * 128, js * 512:(js + 1) * 512])
        wo_sb = []
        for h in range(HPC):
            wot = persist.tile([128, D], BF16, tag=f"wo{h}", name=f"wo_sb{h}")
            dma(wot, wo[h * 128:(h + 1) * 128, :])
            wo_sb.append(wot)

        # ---- persistent activations ----
        qT = [persist.tile([128, T], BF16, tag=f"qT{h}", name=f"qT{h}")
              for h in range(HPC)]
        kT = persist.tile([128, T], BF16, tag="kT", name="kT")
        v_sb = [persist.tile([128, DH], BF16, tag=f"v{t}", name=f"v{t}")
                for t in range(16)]
        oT = [persist.tile([128, T], BF16, tag=f"oT{h}", name=f"oT{h}")
              for h in range(HPC)]

        for j in range(4):
            sl = slice(j * 512, (j + 1) * 512)

            # --- projections for t-slice j ---
            kps = psum.tile([128, 512], F32, tag="acc", bufs=3, name=f"kps{j}")
            for kb in range(16):
                nc.tensor.matmul(out=kps, lhsT=wk_sb[kb], rhs=xT[kb][:, sl],
                                 start=(kb == 0), stop=(kb == 15))
            nc.scalar.activation(out=kT[:, sl], in_=kps,
                                 func=mybir.ActivationFunctionType.Identity,
                                 bias=bk_sb[:, 0:1], scale=1.0)

            for h in range(HPC):
                qps = psum.tile([128, 512], F32, tag="acc", bufs=3,
                                name=f"qps{j}_{h}")
                for kb in range(16):
                    nc.tensor.matmul(out=qps,
                                     lhsT=wq_sb[kb][:, h * 128:(h + 1) * 128],
                                     rhs=xT[kb][:, sl],
                                     start=(kb == 0), stop=(kb == 15))
                nc.scalar.activation(out=qT[h][:, sl], in_=qps,
                                     func=mybir.ActivationFunctionType.Identity,
                                     bias=bq_sb[:, h:h + 1], scale=1.0)

            # VT projection for slice j, then PE-transpose to natural V
            vps = psum.tile([128, 512], F32, tag="acc", bufs=3, name=f"vps{j}")
            for kb in range(16):
                nc.tensor.matmul(out=vps, lhsT=wv_sb[kb], rhs=xT[kb][:, sl],
                                 start=(kb == 0), stop=(kb == 15))
            vt_sb = work.tile([128, 512], BF16, tag="vt", bufs=2,
                              name=f"vt{j}")
            nc.scalar.activation(out=vt_sb, in_=vps,
                                 func=mybir.ActivationFunctionType.Identity,
                                 bias=bv_sb[:, 0:1], scale=1.0)
            vtp = psum.tile([128, 512], BF16, tag="op", bufs=2, name=f"vtp{j}")
            for sub in range(4):
                nc.tensor.transpose(vtp[:, sub * 128:(sub + 1) * 128],
                                    vt_sb[:, sub * 128:(sub + 1) * 128],
                                    ident)
            for sub in range(4):
                nc.vector.tensor_copy(out=v_sb[4 * j + sub],
                                      in_=vtp[:, sub * 128:(sub + 1) * 128])

            # --- attention for all heads, tq-slice j ---
            ntk = 4 * (j + 1)
            for h in range(HPC):
                otps = psum.tile([128, 512], F32, tag="acc", bufs=3,
                                 name=f"otps{h}_{j}")
                racc = work.tile([128, 512], F32, tag="racc", bufs=2,
                                 name=f"racc{h}_{j}")
                for tkb in range(ntk):
                    sps = psum.tile([128, 512], F32, tag="sp", bufs=3,
                                    name=f"sps{h}_{j}_{tkb}")
                    nc.tensor.matmul(out=sps,
                                     lhsT=kT[:, tkb * 128:(tkb + 1) * 128],
                                     rhs=qT[h][:, sl],
                                     start=True, stop=True)
                    pt = work.tile([128, 512], BF16, tag="pt", bufs=6,
                                   name=f"pt{h}_{j}_{tkb}")
                    nc.scalar.activation(out=pt, in_=sps,
                                         func=mybir.ActivationFunctionType.Exp,
                                         scale=SCALE)
                    if tkb >= 4 * j:
                        # causal: zero pt where tq_free < tk_part + 128*r
                        nc.gpsimd.affine_select(
                            out=pt, in_=pt,
                            compare_op=mybir.AluOpType.is_ge,
                            fill=0.0,
                            base=-(128 * (tkb - 4 * j)),
                            pattern=[[1, 512]],
                            channel_multiplier=-1,
                        )
                    nc.tensor.matmul(out=otps, lhsT=v_sb[tkb], rhs=pt,
                                     start=(tkb == 0), stop=(tkb == ntk - 1))
                    if tkb == 0:
                        nc.vector.tensor_copy(out=racc, in_=pt)
                    else:
                        nc.vector.tensor_add(out=racc, in0=racc, in1=pt)
                rsb = psum.tile([128, 512], F32, tag="acc", bufs=3,
                                name=f"rsb{h}_{j}")
                nc.tensor.matmul(out=rsb, lhsT=ones32, rhs=racc,
                                 start=True, stop=True)
                rinv = work.tile([128, 512], F32, tag="rinv", bufs=2,
                                 name=f"rinv{h}_{j}")
                nc.vector.reciprocal_approx_fast(rinv, rsb)
                nc.vector.tensor_mul(out=oT[h][:, sl], in0=otps, in1=rinv)

            # --- output projection for the 4 t-tiles of slice j ---
            for sub in range(4):
                tt = 4 * j + sub
                ostg = work.tile([128, D], F32, tag="ostg", bufs=2,
                                 name=f"ostg{tt}")
                for n in range(4):
                    ops = psum.tile([128, 512], F32, tag="op", bufs=2,
                                    name=f"ops{tt}_{n}")
                    for h in range(HPC):
                        nc.tensor.matmul(
                            out=ops,
                            lhsT=oT[h][:, tt * 128:(tt + 1) * 128],
                            rhs=wo_sb[h][:, n * 512:(n + 1) * 512],
                            start=(h == 0), stop=(h == HPC - 1))
                    nc.vector.tensor_copy(out=ostg[:, n * 512:(n + 1) * 512],
                                          in_=ops)
                nc.sync.dma_start(out=part[tt * 128:(tt + 1) * 128, :],
                                  in_=ostg)

    nc.compile()
    return nc


def _get_nc():
    if "nc" not in _CACHE:
        _CACHE["nc"] = _build_nc()
    return _CACHE["nc"]


def _bf16(a):
    return np.ascontiguousarray(a.astype(ml_dtypes.bfloat16))


def kernel(x, Wq, bq, Wk, bk, Wv, bv, Wo, bo, **kw):
    x = np.asarray(x, dtype=np.float32)
    Wq = np.asarray(Wq, dtype=np.float32)
    Wk = np.asarray(Wk, dtype=np.float32)
    Wv = np.asarray(Wv, dtype=np.float32)
    Wo = np.asarray(Wo, dtype=np.float32)
    bq = np.asarray(bq, dtype=np.float32)
    bk = np.asarray(bk, dtype=np.float32)
    bv = np.asarray(bv, dtype=np.float32)
    bo = np.asarray(bo, dtype=np.float32)

    nc = _get_nc()
    xt_b = [_bf16(x[b].T) for b in range(B)]
    in_maps = []
    for c in range(NCORES):
        b = c // 4
        q = c % 4
        hs = q * HPC * DH          # column start in Wq / row start in Wo
        kv = q // 2
        bq_m = np.ascontiguousarray(
            bq[hs:hs + HPC * DH].reshape(HPC, DH).T)          # [128, 4]
        bk_m = np.ascontiguousarray(
            bk[kv * DH:(kv + 1) * DH].reshape(DH, 1))         # [128, 1]
        bv_m = np.ascontiguousarray(
            bv[kv * DH:(kv + 1) * DH].reshape(DH, 1))         # [128, 1]
        in_maps.append({
            "xt": xt_b[b],
            "wq": _bf16(Wq[:, hs:hs + HPC * DH]),
            "wk": _bf16(Wk[:, kv * DH:(kv + 1) * DH]),
            "wv": _bf16(Wv[:, kv * DH:(kv + 1) * DH]),
            "wo": _bf16(Wo[hs:hs + HPC * DH, :]),
            "bqm": bq_m,
            "bkm": bk_m,
            "bvm": bv_m,
        })

    res = run_bass_kernel_spmd(nc, in_maps, list(range(NCORES)),
                               **kw.get("_run_kwargs", {}))
    if kw.get("_return_res"):
        return res
    parts = [res.results[c]["part"] for c in range(NCORES)]
    out = np.empty((B, T, D), dtype=np.float32)
    for b in range(B):
        acc = parts[4 * b].astype(np.float32).copy()
        for q in range(1, 4):
            acc += parts[4 * b + q]
        out[b] = acc + bo[None, :]
    return out



# revision 3
# speedup vs baseline: 1.1794x; 1.1794x over previous
"""GQA kernel for Trainium2, 8 NeuronCores.

Problem: B=2, T=2048, D=2048, 16 query heads / 2 KV heads, d_head=128, causal.

Sharding: core c -> batch b = c//4, head-quarter q = c%4 (query heads
4q..4q+3, kv head q//2). Each core computes its 4 heads' attention and a
partial output projection (its Wo rows); host sums the 4 partials per batch
and adds bo.

Host marshalling: all inputs pre-cast to bf16 and packed into [128, N]
arrays whose column layout equals the SBUF tile layout, so each logical
group is ONE large DMA with multi-KB contiguous rows, issued on a single
HWDGE queue in exact consumption order:
  wk, x-slice0, wq, wv, x-slice1, wo, x-slice2, x-slice3.

On-core dataflow (bf16 matmuls, fp32 PSUM accum), 4 rounds over 512-wide
t-slices. Per slice j the ACT-heavy attention blocks (score -> exp ->
[causal mask] -> AV) are emitted software-pipelined (score k+2 ahead of
AV k) with the PE-dense filler work (projections of slice j+1, output
projection of slice j-1) spread between them one matmul at a time, so the
statically-ordered PE queue never waits on ACT/DVE progress.

PSUM: sps bufs=3 (tag sp), otps bufs=2 (tag ot, long-lived across the tk
loop), everything else (proj accums, rowsum, oproj, V-transpose) rotates
through tag acc bufs=3.  Row sums accumulate on DVE in bf16 (2x mode);
output partials are written bf16 (host sums in fp32).
"""

import numpy as np
import ml_dtypes
from contextlib import ExitStack

import concourse.bass as bass
from concourse import bacc
import concourse.mybir as mybir
import concourse.tile as tile
from concourse.bass_utils import run_bass_kernel_spmd
from concourse.masks import make_identity

F32 = mybir.dt.float32
BF16 = mybir.dt.bfloat16

D = 2048
T = 2048
DH = 128
B = 2
HPC = 4            # query heads per core
NCORES = 8
NSL = 4            # t-slices of 512
SCALE = 1.0 / float(np.sqrt(128.0))

_CACHE = {}


def _build_nc():
    nc = bacc.Bacc("TRN2", target_bir_lowering=False, debug=False,
                   num_devices=NCORES)

    # packed inputs: one dram tensor per DMA group, rows are the SBUF
    # partition lines (multi-KB contiguous per row)
    xsd = [nc.dram_tensor(f"xs{j}", [128, 16 * 512], BF16,
                          kind="ExternalInput") for j in range(NSL)]
    wkd = nc.dram_tensor("wkx", [128, 16 * 128], BF16, kind="ExternalInput")
    wqd = nc.dram_tensor("wqx", [128, 16 * 512], BF16, kind="ExternalInput")
    wvd = nc.dram_tensor("wvx", [128, 16 * 128], BF16, kind="ExternalInput")
    wod = nc.dram_tensor("wox", [128, HPC * D], BF16, kind="ExternalInput")
    bqm = nc.dram_tensor("bqm", [DH, HPC], F32, kind="ExternalInput")
    bkm = nc.dram_tensor("bkm", [DH, 1], F32, kind="ExternalInput")
    bvm = nc.dram_tensor("bvm", [DH, 1], F32, kind="ExternalInput")
    part = nc.dram_tensor("part", [T, D], BF16, kind="ExternalOutput")

    with ExitStack() as ctx:
        tc = ctx.enter_context(tile.TileContext(nc))
        persist = ctx.enter_context(tc.tile_pool(name="persist", bufs=1))
        work = ctx.enter_context(tc.tile_pool(name="work", bufs=3))
        psum = ctx.enter_context(tc.tile_pool(name="psum", bufs=2, space="PSUM"))

        # ---- constants ----
        ones_bf = persist.tile([128, 128], BF16, tag="ones", name="ones_bf")
        nc.vector.memset(ones_bf, 1.0)
        ident = persist.tile([128, 128], BF16, tag="ident", name="ident")
        make_identity(nc, ident)

        bq_sb = persist.tile([DH, HPC], F32, tag="bq", name="bq_sb")
        nc.scalar.dma_start(out=bq_sb, in_=bqm[:, :])
        bk_sb = persist.tile([DH, 1], F32, tag="bk", name="bk_sb")
        nc.scalar.dma_start(out=bk_sb, in_=bkm[:, :])
        bv_sb = persist.tile([DH, 1], F32, tag="bv", name="bv_sb")
        nc.scalar.dma_start(out=bv_sb, in_=bvm[:, :])

        # ---- inputs -> SBUF, one big DMA per group on the sync HWDGE
        # queue, in exact consumption order ----
        xs_sb = [persist.tile([128, 16 * 512], BF16, tag=f"xs{j}",
                              name=f"xs_sb{j}") for j in range(NSL)]
        wk_sb = persist.tile([128, 16 * 128], BF16, tag="wk", name="wk_sb")
        wq_sb = persist.tile([128, 16 * 512], BF16, tag="wq", name="wq_sb")
        wv_sb = persist.tile([128, 16 * 128], BF16, tag="wv", name="wv_sb")
        wo_sb = persist.tile([128, HPC * D], BF16, tag="wo", name="wo_sb")

        nc.sync.dma_start(out=wk_sb, in_=wkd[:, :])
        nc.sync.dma_start(out=xs_sb[0], in_=xsd[0][:, :])
        nc.sync.dma_start(out=wq_sb, in_=wqd[:, :])
        nc.sync.dma_start(out=wv_sb, in_=wvd[:, :])
        nc.sync.dma_start(out=xs_sb[1], in_=xsd[1][:, :])
        nc.sync.dma_start(out=wo_sb, in_=wod[:, :])
        nc.sync.dma_start(out=xs_sb[2], in_=xsd[2][:, :])
        nc.sync.dma_start(out=xs_sb[3], in_=xsd[3][:, :])

        # ---- persistent activations ----
        qT = [persist.tile([128, T], BF16, tag=f"qT{h}", name=f"qT{h}")
              for h in range(HPC)]
        kT = persist.tile([128, T], BF16, tag="kT", name="kT")
        v_sb = [persist.tile([128, DH], BF16, tag=f"v{t}", name=f"v{t}")
                for t in range(16)]
        oT = [persist.tile([128, T], BF16, tag=f"oT{h}", name=f"oT{h}")
              for h in range(HPC)]

        # ------- emission helpers; each returns a list of closures that
        # emit ONE instruction each (plus tile allocs), used as PE filler
        # interleaved into the attention stream -------

        def gen_proj(j, kind, h=0):
            """Projection group for slice j: 16 accumulating MMs + ACT
            epilogue. kind in {'k','q','v'}."""
            sl = slice(j * 512, (j + 1) * 512)
            st = {}

            def mm(kb):
                def f():
                    if kb == 0:
                        st["ps"] = psum.tile([128, 512], F32, tag="acc",
                                             bufs=3, name=f"{kind}ps{j}_{h}")
                    if kind == "k":
                        lhsT = wk_sb[:, kb * 128:(kb + 1) * 128]
                    elif kind == "v":
                        lhsT = wv_sb[:, kb * 128:(kb + 1) * 128]
                    else:
                        lhsT = wq_sb[:, kb * 512 + h * 128:
                                     kb * 512 + (h + 1) * 128]
                    nc.tensor.matmul(out=st["ps"], lhsT=lhsT,
                                     rhs=xs_sb[j][:, kb * 512:(kb + 1) * 512],
                                     start=(kb == 0), stop=(kb == 15))
                return f

            steps = [mm(kb) for kb in range(16)]

            if kind == "k":
                def epi():
                    nc.scalar.activation(out=kT[:, sl], in_=st["ps"],
                                         func=mybir.ActivationFunctionType.Identity,
                                         bias=bk_sb[:, 0:1], scale=1.0)
                steps.append(epi)
            elif kind == "q":
                def epi():
                    nc.scalar.activation(out=qT[h][:, sl], in_=st["ps"],
                                         func=mybir.ActivationFunctionType.Identity,
                                         bias=bq_sb[:, h:h + 1], scale=1.0)
                steps.append(epi)
            else:
                def epi():
                    st["vt"] = work.tile([128, 512], BF16, tag="vt", bufs=2,
                                         name=f"vt{j}")
                    nc.scalar.activation(out=st["vt"], in_=st["ps"],
                                         func=mybir.ActivationFunctionType.Identity,
                                         bias=bv_sb[:, 0:1], scale=1.0)
                steps.append(epi)

                def tr(sub):
                    def f():
                        if sub == 0:
                            st["vtp"] = psum.tile([128, 512], BF16, tag="acc",
                                                  bufs=3, name=f"vtp{j}")
                        nc.tensor.transpose(
                            st["vtp"][:, sub * 128:(sub + 1) * 128],
                            st["vt"][:, sub * 128:(sub + 1) * 128], ident)
                    return f

                def cp(sub):
                    def f():
                        nc.vector.tensor_copy(
                            out=v_sb[4 * j + sub],
                            in_=st["vtp"][:, sub * 128:(sub + 1) * 128])
                    return f

                for sub in range(4):
                    steps.append(tr(sub))
                steps += [cp(sub) for sub in range(4)]
            return steps

        def gen_oproj(j):
            """Output projection for the 4 t-tiles of slice j.  Each
            512-chunk: 4 accumulating MMs + DVE copy to bf16 staging;
            one output DMA per t-tile on the scalar HWDGE queue."""
            steps = []
            for tt in range(4 * j, 4 * j + 4):
                st = {}

                def chunk_mm(st, tt, n, h):
                    def f():
                        if n == 0 and h == 0:
                            st["ostg"] = work.tile([128, D], BF16, tag="ostg",
                                                   bufs=2, name=f"ostg{tt}")
                        if h == 0:
                            st["ops"] = psum.tile([128, 512], F32, tag="acc",
                                                  bufs=3, name=f"ops{tt}_{n}")
                        nc.tensor.matmul(
                            out=st["ops"],
                            lhsT=oT[h][:, tt * 128:(tt + 1) * 128],
                            rhs=wo_sb[:, h * D + n * 512:h * D + (n + 1) * 512],
                            start=(h == 0), stop=(h == HPC - 1))
                    return f

                def chunk_cp(st, tt, n):
                    def f():
                        nc.vector.tensor_copy(
                            out=st["ostg"][:, n * 512:(n + 1) * 512],
                            in_=st["ops"])
                    return f

                for n in range(4):
                    for h in range(HPC):
                        steps.append(chunk_mm(st, tt, n, h))
                    steps.append(chunk_cp(st, tt, n))

                def out_dma(st, tt):
                    def f():
                        nc.scalar.dma_start(
                            out=part[tt * 128:(tt + 1) * 128, :],
                            in_=st["ostg"])
                    return f

                steps.append(out_dma(st, tt))
            return steps

        # ------- attention for slice j with filler interleave -------

        def emit_attention(j, filler):
            sl = slice(j * 512, (j + 1) * 512)
            ntk = 4 * (j + 1)
            nblocks = HPC * ntk
            bdone = 0

            def pop_filler():
                nonlocal bdone
                bdone += 1
                rem_blocks = nblocks - bdone
                if not filler:
                    return
                if rem_blocks <= 0:
                    while filler:
                        filler.pop(0)()
                    return
                k = (len(filler) + rem_blocks - 1) // rem_blocks
                for _ in range(min(k, len(filler))):
                    filler.pop(0)()

            for h in range(HPC):
                st = {"sps": {}, "pt": {}}

                def score(tkb):
                    sps = psum.tile([128, 512], F32, tag="sp", bufs=3,
                                    name=f"sps{j}_{h}_{tkb}")
                    nc.tensor.matmul(out=sps,
                                     lhsT=kT[:, tkb * 128:(tkb + 1) * 128],
                                     rhs=qT[h][:, sl],
                                     start=True, stop=True)
                    pt = work.tile([128, 512], BF16, tag="pt", bufs=6,
                                   name=f"pt{j}_{h}_{tkb}")
                    nc.scalar.activation(out=pt, in_=sps,
                                         func=mybir.ActivationFunctionType.Exp,
                                         scale=SCALE)
                    if tkb >= 4 * j:
                        nc.gpsimd.affine_select(
                            out=pt, in_=pt,
                            compare_op=mybir.AluOpType.is_ge,
                            fill=0.0,
                            base=-(128 * (tkb - 4 * j)),
                            pattern=[[1, 512]],
                            channel_multiplier=-1,
                        )
                    st["pt"][tkb] = pt

                def consume(tkb):
                    pt = st["pt"].pop(tkb)
                    if tkb == 0:
                        st["otps"] = psum.tile([128, 512], F32, tag="ot",
                                               bufs=2, name=f"otps{j}_{h}")
                        st["racc"] = work.tile([128, 512], BF16, tag="racc",
                                               bufs=2, name=f"racc{j}_{h}")
                    nc.tensor.matmul(out=st["otps"], lhsT=v_sb[tkb], rhs=pt,
                                     start=(tkb == 0), stop=(tkb == ntk - 1))
                    if tkb == 0:
                        nc.vector.tensor_copy(out=st["racc"], in_=pt)
                    else:
                        nc.vector.tensor_add(out=st["racc"], in0=st["racc"],
                                             in1=pt)

                # software pipeline: score k+2 runs ahead of AV k
                for tkb in range(ntk):
                    score(tkb)
                    if tkb >= 2:
                        consume(tkb - 2)
                    pop_filler()
                for tkb in range(max(0, ntk - 2), ntk):
                    consume(tkb)

                # finalize head: rowsum via ones-matmul, normalize
                rsb = psum.tile([128, 512], F32, tag="acc", bufs=3,
                                name=f"rsb{j}_{h}")
                nc.tensor.matmul(out=rsb, lhsT=ones_bf, rhs=st["racc"],
                                 start=True, stop=True)
                rinv = work.tile([128, 512], F32, tag="rinv", bufs=2,
                                 name=f"rinv{j}_{h}")
                nc.vector.reciprocal_approx_fast(rinv, rsb)
                nc.vector.tensor_mul(out=oT[h][:, sl], in0=st["otps"],
                                     in1=rinv)

            # leftover filler (normally consumed inside the loop)
            while filler:
                filler.pop(0)()

        # ------- program -------
        # prologue: slice-0 projections, directly
        for f in (gen_proj(0, "k") + gen_proj(0, "q", 0) + gen_proj(0, "q", 1)
                  + gen_proj(0, "q", 2) + gen_proj(0, "q", 3)
                  + gen_proj(0, "v")):
            f()

        for j in range(NSL):
            filler = []
            if j + 1 < NSL:
                filler += gen_proj(j + 1, "k")
                for h in range(HPC):
                    filler += gen_proj(j + 1, "q", h)
                filler += gen_proj(j + 1, "v")
            if j - 1 >= 0:
                filler += gen_oproj(j - 1)
            emit_attention(j, filler)

        # epilogue: last slice's output projection
        for f in gen_oproj(NSL - 1):
            f()

    nc.compile()
    return nc


def _get_nc():
    if "nc" not in _CACHE:
        _CACHE["nc"] = _build_nc()
    return _CACHE["nc"]


def _bf16(a):
    return np.ascontiguousarray(a.astype(ml_dtypes.bfloat16))


def kernel(x, Wq, bq, Wk, bk, Wv, bv, Wo, bo, **kw):
    x = np.asarray(x, dtype=np.float32)
    Wq = np.asarray(Wq, dtype=np.float32)
    Wk = np.asarray(Wk, dtype=np.float32)
    Wv = np.asarray(Wv, dtype=np.float32)
    Wo = np.asarray(Wo, dtype=np.float32)
    bq = np.asarray(bq, dtype=np.float32)
    bk = np.asarray(bk, dtype=np.float32)
    bv = np.asarray(bv, dtype=np.float32)
    bo = np.asarray(bo, dtype=np.float32)

    nc = _get_nc()

    # x slices, shared per batch: xs[b][j] = [128, 16*512] with columns
    # (kb, t') st. xs[b][j][p, kb*512+t'] = x[b, j*512+t', kb*128+p]
    xs_b = []
    for b in range(B):
        xT = np.ascontiguousarray(x[b].T)            # [D, T]
        xs = xT.reshape(16, 128, NSL, 512).transpose(2, 1, 0, 3)
        xs_b.append(_bf16(xs.reshape(NSL, 128, 16 * 512)))

    # per head-quarter weight packs, shared across batches
    packs = []
    for q in range(HPC):
        hs = q * HPC * DH
        kv = q // 2
        wqp = _bf16(Wq[:, hs:hs + HPC * DH].reshape(16, 128, HPC * DH)
                    .transpose(1, 0, 2).reshape(128, 16 * 512))
        wkp = _bf16(Wk[:, kv * DH:(kv + 1) * DH].reshape(16, 128, DH)
                    .transpose(1, 0, 2).reshape(128, 16 * 128))
        wvp = _bf16(Wv[:, kv * DH:(kv + 1) * DH].reshape(16, 128, DH)
                    .transpose(1, 0, 2).reshape(128, 16 * 128))
        wop = _bf16(Wo[hs:hs + HPC * DH, :].reshape(HPC, 128, D)
                    .transpose(1, 0, 2).reshape(128, HPC * D))
        bq_m = np.ascontiguousarray(
            bq[hs:hs + HPC * DH].reshape(HPC, DH).T)          # [128, 4]
        bk_m = np.ascontiguousarray(
            bk[kv * DH:(kv + 1) * DH].reshape(DH, 1))         # [128, 1]
        bv_m = np.ascontiguousarray(
            bv[kv * DH:(kv + 1) * DH].reshape(DH, 1))         # [128, 1]
        packs.append((wqp, wkp, wvp, wop, bq_m, bk_m, bv_m))

    in_maps = []
    for c in range(NCORES):
        b = c // 4
        q = c % 4
        wqp, wkp, wvp, wop, bq_m, bk_m, bv_m = packs[q]
        m = {f"xs{j}": xs_b[b][j] for j in range(NSL)}
        m.update({
            "wqx": wqp, "wkx": wkp, "wvx": wvp, "wox": wop,
            "bqm": bq_m, "bkm": bk_m, "bvm": bv_m,
        })
        in_maps.append(m)

    res = run_bass_kernel_spmd(nc, in_maps, list(range(NCORES)),
                               **kw.get("_run_kwargs", {}))
    if kw.get("_return_res"):
        return res
    parts = [res.results[c]["part"] for c in range(NCORES)]
    out = np.empty((B, T, D), dtype=np.float32)
    for b in range(B):
        acc = parts[4 * b].astype(np.float32)
        for q in range(1, 4):
            acc = acc + parts[4 * b + q].astype(np.float32)
        out[b] = acc + bo[None, :]
    return out


# revision 8
# speedup vs baseline: 1.2073x; 1.0237x over previous
"""GQA kernel for Trainium2, 8 NeuronCores.

Problem: B=2, T=2048, D=2048, 16 query heads / 2 KV heads, d_head=128, causal.

Sharding: core c -> batch b = c//4, head-quarter q = c%4 (query heads
4q..4q+3, kv head q//2). Each core computes its 4 heads' attention and a
partial output projection (its Wo rows); host sums the 4 partials per batch
and adds bo.

Host marshalling: all inputs pre-cast to bf16 and packed into [128, N]
arrays whose column layout equals the SBUF tile layout, so each logical
group is ONE large DMA with multi-KB contiguous rows, issued on a single
HWDGE queue in exact consumption order:
  wk, x-slice0, wq, wv, x-slice1, wo, x-slice2, x-slice3.

On-core dataflow (bf16 matmuls, fp32 PSUM accum), 4 rounds over 512-wide
t-slices. Per slice j the ACT-heavy attention blocks (score -> exp ->
[causal mask] -> AV) are emitted software-pipelined (score k+2 ahead of
AV k) with the PE-dense filler work (projections of slice j+1, output
projection of slice j-1) spread between them one matmul at a time, so the
statically-ordered PE queue never waits on ACT/DVE progress.

PSUM: sps bufs=3 (tag sp), otps bufs=2 (tag ot, long-lived across the tk
loop), everything else (proj accums, rowsum, oproj, V-transpose) rotates
through tag acc bufs=3.  Row sums accumulate on DVE in bf16 (2x mode);
output partials are written bf16 (host sums in fp32).
"""

import numpy as np
import ml_dtypes
from contextlib import ExitStack

import concourse.bass as bass
from concourse import bacc
import concourse.mybir as mybir
import concourse.tile as tile
from concourse.bass_utils import run_bass_kernel_spmd
from concourse.masks import make_identity

F32 = mybir.dt.float32
BF16 = mybir.dt.bfloat16

D = 2048
T = 2048
DH = 128
B = 2
HPC = 4            # query heads per core
NCORES = 8
NSL = 4            # t-slices of 512
SCALE = 1.0 / float(np.sqrt(128.0))

_CACHE = {}


def _build_nc():
    nc = bacc.Bacc("TRN2", target_bir_lowering=False, debug=False,
                   num_devices=NCORES)

    # packed inputs: one dram tensor per DMA group, rows are the SBUF
    # partition lines (multi-KB contiguous per row)
    xsd = [nc.dram_tensor(f"xs{j}", [128, 16 * 512], BF16,
                          kind="ExternalInput") for j in range(NSL)]
    wkd = nc.dram_tensor("wkx", [128, 16 * 128], BF16, kind="ExternalInput")
    wqd = nc.dram_tensor("wqx", [128, 16 * 512], BF16, kind="ExternalInput")
    wvd = nc.dram_tensor("wvx", [128, 16 * 128], BF16, kind="ExternalInput")
    wod = nc.dram_tensor("wox", [128, HPC * D], BF16, kind="ExternalInput")
    bqm = nc.dram_tensor("bqm", [DH, HPC], F32, kind="ExternalInput")
    bkm = nc.dram_tensor("bkm", [DH, 1], F32, kind="ExternalInput")
    bvm = nc.dram_tensor("bvm", [DH, 1], F32, kind="ExternalInput")
    part = nc.dram_tensor("part", [T, D], BF16, kind="ExternalOutput")

    with ExitStack() as ctx:
        tc = ctx.enter_context(tile.TileContext(nc))
        persist = ctx.enter_context(tc.tile_pool(name="persist", bufs=1))
        work = ctx.enter_context(tc.tile_pool(name="work", bufs=3))
        psum = ctx.enter_context(tc.tile_pool(name="psum", bufs=2, space="PSUM"))

        # ---- constants ----
        ones_bf = persist.tile([128, 128], BF16, tag="ones", name="ones_bf")
        nc.vector.memset(ones_bf, 1.0)
        ident = persist.tile([128, 128], BF16, tag="ident", name="ident")
        make_identity(nc, ident)

        bq_sb = persist.tile([DH, HPC], F32, tag="bq", name="bq_sb")
        nc.scalar.dma_start(out=bq_sb, in_=bqm[:, :])
        bk_sb = persist.tile([DH, 1], F32, tag="bk", name="bk_sb")
        nc.scalar.dma_start(out=bk_sb, in_=bkm[:, :])
        bv_sb = persist.tile([DH, 1], F32, tag="bv", name="bv_sb")
        nc.scalar.dma_start(out=bv_sb, in_=bvm[:, :])

        # ---- inputs -> SBUF on the sync HWDGE queue, in exact consumption
        # order.  Slice-0 groups are split fine-grained so the first
        # projection matmuls start as early as possible. ----
        xs0_sb = [persist.tile([128, 4 * 512], BF16, tag=f"xs0_{g}",
                               name=f"xs0_sb{g}") for g in range(4)]
        xs_sb = [None] + [persist.tile([128, 16 * 512], BF16, tag=f"xs{j}",
                                       name=f"xs_sb{j}") for j in range(1, NSL)]
        wk_sb = persist.tile([128, 16 * 128], BF16, tag="wk", name="wk_sb")
        wq2_sb = [persist.tile([128, 8 * 512], BF16, tag=f"wq{g}",
                               name=f"wq_sb{g}") for g in range(2)]
        wv_sb = persist.tile([128, 16 * 128], BF16, tag="wv", name="wv_sb")
        wo_sb = persist.tile([128, HPC * D], BF16, tag="wo", name="wo_sb")

        nc.sync.dma_start(out=wk_sb, in_=wkd[:, :])
        for g in range(4):
            nc.sync.dma_start(out=xs0_sb[g],
                              in_=xsd[0][:, g * 2048:(g + 1) * 2048])
        for g in range(2):
            nc.sync.dma_start(out=wq2_sb[g],
                              in_=wqd[:, g * 4096:(g + 1) * 4096])
        nc.sync.dma_start(out=wv_sb, in_=wvd[:, :])
        nc.sync.dma_start(out=xs_sb[1], in_=xsd[1][:, :])
        nc.sync.dma_start(out=wo_sb, in_=wod[:, :])
        nc.sync.dma_start(out=xs_sb[2], in_=xsd[2][:, :])
        nc.sync.dma_start(out=xs_sb[3], in_=xsd[3][:, :])

        def xrhs(j, kb):
            if j == 0:
                return xs0_sb[kb // 4][:, (kb % 4) * 512:(kb % 4 + 1) * 512]
            return xs_sb[j][:, kb * 512:(kb + 1) * 512]

        def wqap(kb, h):
            return wq2_sb[kb // 8][:, (kb % 8) * 512 + h * 128:
                                   (kb % 8) * 512 + (h + 1) * 128]

        # ---- persistent activations ----
        qT = [persist.tile([128, T], BF16, tag=f"qT{h}", name=f"qT{h}")
              for h in range(HPC)]
        kT = persist.tile([128, T], BF16, tag="kT", name="kT")
        v_sb = [persist.tile([128, DH], BF16, tag=f"v{t}", name=f"v{t}")
                for t in range(16)]
        oT = [persist.tile([128, T], BF16, tag=f"oT{h}", name=f"oT{h}")
              for h in range(HPC)]

        # ------- emission helpers; each returns a list of closures that
        # emit ONE instruction each (plus tile allocs), used as PE filler
        # interleaved into the attention stream -------

        def gen_proj(j, kind, h=0):
            """Projection group for slice j: 16 accumulating MMs + ACT
            epilogue. kind in {'k','q','v'}."""
            sl = slice(j * 512, (j + 1) * 512)
            st = {}

            def mm(kb):
                def f():
                    if kb == 0:
                        st["ps"] = psum.tile([128, 512], F32, tag="acc",
                                             bufs=3, name=f"{kind}ps{j}_{h}")
                    if kind == "k":
                        lhsT = wk_sb[:, kb * 128:(kb + 1) * 128]
                    elif kind == "v":
                        lhsT = wv_sb[:, kb * 128:(kb + 1) * 128]
                    else:
                        lhsT = wqap(kb, h)
                    nc.tensor.matmul(out=st["ps"], lhsT=lhsT,
                                     rhs=xrhs(j, kb),
                                     start=(kb == 0), stop=(kb == 15))
                return f

            steps = [mm(kb) for kb in range(16)]

            if kind == "k":
                def epi():
                    nc.scalar.activation(out=kT[:, sl], in_=st["ps"],
                                         func=mybir.ActivationFunctionType.Identity,
                                         bias=bk_sb[:, 0:1], scale=1.0)
                steps.append(epi)
            elif kind == "q":
                def epi():
                    nc.scalar.activation(out=qT[h][:, sl], in_=st["ps"],
                                         func=mybir.ActivationFunctionType.Identity,
                                         bias=bq_sb[:, h:h + 1], scale=1.0)
                steps.append(epi)
            else:
                def epi():
                    st["vt"] = work.tile([128, 512], BF16, tag="vt", bufs=2,
                                         name=f"vt{j}")
                    nc.scalar.activation(out=st["vt"], in_=st["ps"],
                                         func=mybir.ActivationFunctionType.Identity,
                                         bias=bv_sb[:, 0:1], scale=1.0)
                steps.append(epi)

                def tr(sub):
                    def f():
                        if sub == 0:
                            st["vtp"] = psum.tile([128, 512], BF16, tag="acc",
                                                  bufs=3, name=f"vtp{j}")
                        nc.tensor.transpose(
                            st["vtp"][:, sub * 128:(sub + 1) * 128],
                            st["vt"][:, sub * 128:(sub + 1) * 128], ident)
                    return f

                def cp(sub):
                    def f():
                        nc.vector.tensor_copy(
                            out=v_sb[4 * j + sub],
                            in_=st["vtp"][:, sub * 128:(sub + 1) * 128])
                    return f

                for sub in range(4):
                    steps.append(tr(sub))
                steps += [cp(sub) for sub in range(4)]
            return steps

        def gen_oproj(j, tiles=range(4)):
            """Output projection for t-tiles of slice j.  Each 512-chunk:
            4 accumulating MMs + DVE copy to bf16 staging; chunked output
            DMAs on the scalar HWDGE queue."""
            steps = []
            for tt in [4 * j + s for s in tiles]:
                st = {}

                def chunk_mm(st, tt, n, h):
                    def f():
                        if n == 0 and h == 0:
                            st["ostg"] = work.tile([128, D], BF16, tag="ostg",
                                                   bufs=2, name=f"ostg{tt}")
                        if h == 0:
                            st["ops"] = psum.tile([128, 512], F32, tag="acc",
                                                  bufs=3, name=f"ops{tt}_{n}")
                        nc.tensor.matmul(
                            out=st["ops"],
                            lhsT=oT[h][:, tt * 128:(tt + 1) * 128],
                            rhs=wo_sb[:, h * D + n * 512:h * D + (n + 1) * 512],
                            start=(h == 0), stop=(h == HPC - 1))
                    return f

                def chunk_cp(st, tt, n):
                    def f():
                        nc.vector.tensor_copy(
                            out=st["ostg"][:, n * 512:(n + 1) * 512],
                            in_=st["ops"])
                    return f

                def out_dma(st, tt, half):
                    def f():
                        nc.scalar.dma_start(
                            out=part[tt * 128:(tt + 1) * 128,
                                     half * 1024:(half + 1) * 1024],
                            in_=st["ostg"][:, half * 1024:(half + 1) * 1024])
                    return f

                for n in range(4):
                    for h in range(HPC):
                        steps.append(chunk_mm(st, tt, n, h))
                    steps.append(chunk_cp(st, tt, n))
                    if n % 2 == 1:
                        steps.append(out_dma(st, tt, n // 2))
            return steps

        # ------- attention for slice j with filler interleave -------

        def emit_attention(j, filler):
            sl = slice(j * 512, (j + 1) * 512)
            ntk = 4 * (j + 1)
            nblocks = HPC * ntk
            bdone = 0

            def pop_filler():
                nonlocal bdone
                bdone += 1
                rem_blocks = nblocks - bdone
                if not filler:
                    return
                if rem_blocks <= 0:
                    while filler:
                        filler.pop(0)()
                    return
                k = (len(filler) + rem_blocks - 1) // rem_blocks
                for _ in range(min(k, len(filler))):
                    filler.pop(0)()

            for h in range(HPC):
                st = {"sps": {}, "pt": {}}

                def score(tkb):
                    sps = psum.tile([128, 512], F32, tag="sp", bufs=3,
                                    name=f"sps{j}_{h}_{tkb}")
                    nc.tensor.matmul(out=sps,
                                     lhsT=kT[:, tkb * 128:(tkb + 1) * 128],
                                     rhs=qT[h][:, sl],
                                     start=True, stop=True)
                    pt = work.tile([128, 512], BF16, tag="pt", bufs=6,
                                   name=f"pt{j}_{h}_{tkb}")
                    nc.scalar.activation(out=pt, in_=sps,
                                         func=mybir.ActivationFunctionType.Exp,
                                         scale=SCALE)
                    if tkb >= 4 * j:
                        nc.gpsimd.affine_select(
                            out=pt, in_=pt,
                            compare_op=mybir.AluOpType.is_ge,
                            fill=0.0,
                            base=-(128 * (tkb - 4 * j)),
                            pattern=[[1, 512]],
                            channel_multiplier=-1,
                        )
                    st["pt"][tkb] = pt

                def consume(tkb):
                    pt = st["pt"].pop(tkb)
                    if tkb == 0:
                        st["otps"] = psum.tile([128, 512], F32, tag="ot",
                                               bufs=2, name=f"otps{j}_{h}")
                        st["racc"] = work.tile([128, 512], BF16, tag="racc",
                                               bufs=2, name=f"racc{j}_{h}")
                    nc.tensor.matmul(out=st["otps"], lhsT=v_sb[tkb], rhs=pt,
                                     start=(tkb == 0), stop=(tkb == ntk - 1))
                    if tkb == 0:
                        nc.vector.tensor_copy(out=st["racc"], in_=pt)
                    else:
                        nc.vector.tensor_add(out=st["racc"], in0=st["racc"],
                                             in1=pt)

                # software pipeline: score k+2 runs ahead of AV k
                for tkb in range(ntk):
                    score(tkb)
                    if tkb >= 2:
                        consume(tkb - 2)
                    pop_filler()
                for tkb in range(max(0, ntk - 2), ntk):
                    consume(tkb)

                # finalize head: rowsum via ones-matmul, normalize
                rsb = psum.tile([128, 512], F32, tag="acc", bufs=3,
                                name=f"rsb{j}_{h}")
                nc.tensor.matmul(out=rsb, lhsT=ones_bf, rhs=st["racc"],
                                 start=True, stop=True)
                rinv = work.tile([128, 512], F32, tag="rinv", bufs=2,
                                 name=f"rinv{j}_{h}")
                nc.vector.reciprocal_approx_fast(rinv, rsb)
                nc.vector.tensor_mul(out=oT[h][:, sl], in0=st["otps"],
                                     in1=rinv)

            # leftover filler (normally consumed inside the loop)
            while filler:
                filler.pop(0)()

        # ------- program -------
        # prologue: slice-0 projections, directly
        for f in (gen_proj(0, "k") + gen_proj(0, "q", 0) + gen_proj(0, "q", 1)
                  + gen_proj(0, "q", 2) + gen_proj(0, "q", 3)
                  + gen_proj(0, "v")):
            f()

        # filler distribution: defer half of oproj(1) to slice 3 so the
        # long slice-3 attention keeps enough PE filler
        for j in range(NSL):
            filler = []
            if j + 1 < NSL:
                filler += gen_proj(j + 1, "k")
                for h in range(HPC):
                    filler += gen_proj(j + 1, "q", h)
                filler += gen_proj(j + 1, "v")
            if j == 1:
                filler += gen_oproj(0)
            elif j == 2:
                filler += gen_oproj(1, tiles=(0, 1))
            elif j == 3:
                filler += gen_oproj(1, tiles=(2, 3))
                filler += gen_oproj(2)
            emit_attention(j, filler)

        # epilogue: last slice's output projection
        for f in gen_oproj(NSL - 1):
            f()

    nc.compile()
    return nc


def _get_nc():
    if "nc" not in _CACHE:
        _CACHE["nc"] = _build_nc()
    return _CACHE["nc"]


def _bf16(a):
    return np.ascontiguousarray(a.astype(ml_dtypes.bfloat16))


def kernel(x, Wq, bq, Wk, bk, Wv, bv, Wo, bo, **kw):
    x = np.asarray(x, dtype=np.float32)
    Wq = np.asarray(Wq, dtype=np.float32)
    Wk = np.asarray(Wk, dtype=np.float32)
    Wv = np.asarray(Wv, dtype=np.float32)
    Wo = np.asarray(Wo, dtype=np.float32)
    bq = np.asarray(bq, dtype=np.float32)
    bk = np.asarray(bk, dtype=np.float32)
    bv = np.asarray(bv, dtype=np.float32)
    bo = np.asarray(bo, dtype=np.float32)

    nc = _get_nc()

    # x slices, shared per batch: xs[b][j] = [128, 16*512] with columns
    # (kb, t') st. xs[b][j][p, kb*512+t'] = x[b, j*512+t', kb*128+p]
    xs_b = []
    for b in range(B):
        xT = np.ascontiguousarray(x[b].T)            # [D, T]
        xs = xT.reshape(16, 128, NSL, 512).transpose(2, 1, 0, 3)
        xs_b.append(_bf16(xs.reshape(NSL, 128, 16 * 512)))

    # per head-quarter weight packs, shared across batches
    packs = []
    for q in range(HPC):
        hs = q * HPC * DH
        kv = q // 2
        wqp = _bf16(Wq[:, hs:hs + HPC * DH].reshape(16, 128, HPC * DH)
                    .transpose(1, 0, 2).reshape(128, 16 * 512))
        wkp = _bf16(Wk[:, kv * DH:(kv + 1) * DH].reshape(16, 128, DH)
                    .transpose(1, 0, 2).reshape(128, 16 * 128))
        wvp = _bf16(Wv[:, kv * DH:(kv + 1) * DH].reshape(16, 128, DH)
                    .transpose(1, 0, 2).reshape(128, 16 * 128))
        wop = _bf16(Wo[hs:hs + HPC * DH, :].reshape(HPC, 128, D)
                    .transpose(1, 0, 2).reshape(128, HPC * D))
        bq_m = np.ascontiguousarray(
            bq[hs:hs + HPC * DH].reshape(HPC, DH).T)          # [128, 4]
        bk_m = np.ascontiguousarray(
            bk[kv * DH:(kv + 1) * DH].reshape(DH, 1))         # [128, 1]
        bv_m = np.ascontiguousarray(
            bv[kv * DH:(kv + 1) * DH].reshape(DH, 1))         # [128, 1]
        packs.append((wqp, wkp, wvp, wop, bq_m, bk_m, bv_m))

    in_maps = []
    for c in range(NCORES):
        b = c // 4
        q = c % 4
        wqp, wkp, wvp, wop, bq_m, bk_m, bv_m = packs[q]
        m = {f"xs{j}": xs_b[b][j] for j in range(NSL)}
        m.update({
            "wqx": wqp, "wkx": wkp, "wvx": wvp, "wox": wop,
            "bqm": bq_m, "bkm": bk_m, "bvm": bv_m,
        })
        in_maps.append(m)

    res = run_bass_kernel_spmd(nc, in_maps, list(range(NCORES)),
                               **kw.get("_run_kwargs", {}))
    if kw.get("_return_res"):
        return res
    parts = [res.results[c]["part"] for c in range(NCORES)]
    out = np.empty((B, T, D), dtype=np.float32)
    for b in range(B):
        acc = parts[4 * b].astype(np.float32)
        for q in range(1, 4):
            acc = acc + parts[4 * b + q].astype(np.float32)
        out[b] = acc + bo[None, :]
    return out


# revision 13
# speedup vs baseline: 1.4115x; 1.1691x over previous
"""GQA kernel for Trainium2, 8 NeuronCores.

Problem: B=2, T=2048, D=2048, 16 query heads / 2 KV heads, d_head=128, causal.

Sharding: core c -> batch b = c//4, head-quarter q = c%4 (query heads
4q..4q+3, kv head q//2). Each core computes its 4 heads' attention and a
partial output projection (its Wo rows); host sums the 4 partials per batch
and adds bo.

Host marshalling: all inputs pre-cast to bf16 and packed into [128, N]
arrays whose column layout equals the SBUF tile layout, so each logical
group is ONE large DMA with multi-KB contiguous rows, issued on a single
HWDGE queue in exact consumption order:
  wk, x-slice0, wq, wv, x-slice1, wo, x-slice2, x-slice3.

On-core dataflow (bf16 matmuls, fp32 PSUM accum), 4 rounds over 512-wide
t-slices. Per slice j the ACT-heavy attention blocks (score -> exp ->
[causal mask] -> AV) are emitted software-pipelined (score k+2 ahead of
AV k) with the PE-dense filler work (projections of slice j+1, output
projection of slice j-1) spread between them one matmul at a time, so the
statically-ordered PE queue never waits on ACT/DVE progress.

PSUM: sps bufs=3 (tag sp), otps bufs=2 (tag ot, long-lived across the tk
loop), everything else (proj accums, rowsum, oproj, V-transpose) rotates
through tag acc bufs=3.  Row sums accumulate on DVE in bf16 (2x mode);
output partials are written bf16 (host sums in fp32).
"""

import numpy as np
import ml_dtypes
from contextlib import ExitStack

import concourse.bass as bass
from concourse import bacc
import concourse.mybir as mybir
import concourse.tile as tile
from concourse.bass_utils import run_bass_kernel_spmd
from concourse.masks import make_identity

F32 = mybir.dt.float32
BF16 = mybir.dt.bfloat16

D = 2048
T = 2048
DH = 128
B = 2
HPC = 4            # query heads per core
NCORES = 8
NSL = 4            # t-slices of 512
SCALE = 1.0 / float(np.sqrt(128.0))

_CACHE = {}


def _build_nc():
    nc = bacc.Bacc("TRN2", target_bir_lowering=False, debug=False,
                   num_devices=NCORES)

    # packed inputs: one dram tensor per DMA group, rows are the SBUF
    # partition lines (multi-KB contiguous per row)
    xsd = [nc.dram_tensor(f"xs{j}", [128, 16 * 512], BF16,
                          kind="ExternalInput") for j in range(NSL)]
    wkd = nc.dram_tensor("wkx", [128, 16 * 128], BF16, kind="ExternalInput")
    wqd = nc.dram_tensor("wqx", [128, 16 * 512], BF16, kind="ExternalInput")
    wvd = nc.dram_tensor("wvx", [128, 16 * 128], BF16, kind="ExternalInput")
    wod = nc.dram_tensor("wox", [128, HPC * D], BF16, kind="ExternalInput")
    bqm = nc.dram_tensor("bqm", [DH, HPC], F32, kind="ExternalInput")
    bkm = nc.dram_tensor("bkm", [DH, 1], F32, kind="ExternalInput")
    bvm = nc.dram_tensor("bvm", [DH, 1], F32, kind="ExternalInput")
    part = nc.dram_tensor("part", [T, D], BF16, kind="ExternalOutput")

    with ExitStack() as ctx:
        tc = ctx.enter_context(tile.TileContext(nc))
        persist = ctx.enter_context(tc.tile_pool(name="persist", bufs=1))
        work = ctx.enter_context(tc.tile_pool(name="work", bufs=3))
        psum = ctx.enter_context(tc.tile_pool(name="psum", bufs=2, space="PSUM"))

        # ---- constants ----
        ones_bf = persist.tile([128, 128], BF16, tag="ones", name="ones_bf")
        nc.vector.memset(ones_bf, 1.0)
        ident = persist.tile([128, 128], BF16, tag="ident", name="ident")
        make_identity(nc, ident)

        bq_sb = persist.tile([DH, HPC], F32, tag="bq", name="bq_sb")
        nc.scalar.dma_start(out=bq_sb, in_=bqm[:, :])
        bk_sb = persist.tile([DH, 1], F32, tag="bk", name="bk_sb")
        nc.scalar.dma_start(out=bk_sb, in_=bkm[:, :])
        bv_sb = persist.tile([DH, 1], F32, tag="bv", name="bv_sb")
        nc.scalar.dma_start(out=bv_sb, in_=bvm[:, :])

        # ---- inputs -> SBUF on the sync HWDGE queue, in exact consumption
        # order.  Slice-0 groups are split fine-grained so the first
        # projection matmuls start as early as possible. ----
        xs0_sb = [persist.tile([128, 4 * 512], BF16, tag=f"xs0_{g}",
                               name=f"xs0_sb{g}") for g in range(4)]
        xs_sb = [None] + [persist.tile([128, 16 * 512], BF16, tag=f"xs{j}",
                                       name=f"xs_sb{j}") for j in range(1, NSL)]
        wk_sb = persist.tile([128, 16 * 128], BF16, tag="wk", name="wk_sb")
        wq2_sb = [persist.tile([128, 8 * 512], BF16, tag=f"wq{g}",
                               name=f"wq_sb{g}") for g in range(2)]
        wv_sb = persist.tile([128, 16 * 128], BF16, tag="wv", name="wv_sb")
        wo_sb = persist.tile([128, HPC * D], BF16, tag="wo", name="wo_sb")

        nc.sync.dma_start(out=wk_sb[:, 0:512], in_=wkd[:, 0:512])
        nc.sync.dma_start(out=wk_sb[:, 512:2048], in_=wkd[:, 512:2048])
        for g in range(4):
            nc.sync.dma_start(out=xs0_sb[g],
                              in_=xsd[0][:, g * 2048:(g + 1) * 2048])
        nc.sync.dma_start(out=wv_sb, in_=wvd[:, :])
        for g in range(2):
            nc.sync.dma_start(out=wq2_sb[g],
                              in_=wqd[:, g * 4096:(g + 1) * 4096])
        nc.sync.dma_start(out=xs_sb[1], in_=xsd[1][:, :])
        nc.sync.dma_start(out=wo_sb, in_=wod[:, :])
        nc.sync.dma_start(out=xs_sb[2], in_=xsd[2][:, :])
        nc.sync.dma_start(out=xs_sb[3], in_=xsd[3][:, :])

        def xrhs(j, kb):
            if j == 0:
                return xs0_sb[kb // 4][:, (kb % 4) * 512:(kb % 4 + 1) * 512]
            return xs_sb[j][:, kb * 512:(kb + 1) * 512]

        def wqap(kb, h):
            return wq2_sb[kb // 8][:, (kb % 8) * 512 + h * 128:
                                   (kb % 8) * 512 + (h + 1) * 128]

        # ---- persistent activations ----
        qT = [persist.tile([128, T], BF16, tag=f"qT{h}", name=f"qT{h}")
              for h in range(HPC)]
        kT = persist.tile([128, T], BF16, tag="kT", name="kT")
        v_sb = [persist.tile([128, DH], BF16, tag=f"v{t}", name=f"v{t}")
                for t in range(16)]
        oT = [persist.tile([128, T], BF16, tag=f"oT{h}", name=f"oT{h}")
              for h in range(HPC)]

        # ------- emission helpers; each returns a list of closures that
        # emit ONE instruction each (plus tile allocs), used as PE filler
        # interleaved into the attention stream -------

        def gen_proj(j, kind, h=0):
            """Projection group for slice j: 16 accumulating MMs + ACT
            epilogue. kind in {'k','q','v'}."""
            sl = slice(j * 512, (j + 1) * 512)
            st = {}

            def mm(kb):
                def f():
                    if kb == 0:
                        st["ps"] = psum.tile([128, 512], F32, tag="acc",
                                             bufs=3, name=f"{kind}ps{j}_{h}")
                    if kind == "k":
                        lhsT = wk_sb[:, kb * 128:(kb + 1) * 128]
                    elif kind == "v":
                        lhsT = wv_sb[:, kb * 128:(kb + 1) * 128]
                    else:
                        lhsT = wqap(kb, h)
                    nc.tensor.matmul(out=st["ps"], lhsT=lhsT,
                                     rhs=xrhs(j, kb),
                                     start=(kb == 0), stop=(kb == 15))
                return f

            steps = [mm(kb) for kb in range(16)]

            if kind == "k":
                def epi():
                    nc.scalar.activation(out=kT[:, sl], in_=st["ps"],
                                         func=mybir.ActivationFunctionType.Identity,
                                         bias=bk_sb[:, 0:1], scale=1.0)
                steps.append(epi)
            elif kind == "q":
                def epi():
                    nc.scalar.activation(out=qT[h][:, sl], in_=st["ps"],
                                         func=mybir.ActivationFunctionType.Identity,
                                         bias=bq_sb[:, h:h + 1], scale=1.0)
                steps.append(epi)
            else:
                def epi():
                    st["vt"] = work.tile([128, 512], BF16, tag="vt", bufs=2,
                                         name=f"vt{j}")
                    nc.scalar.activation(out=st["vt"], in_=st["ps"],
                                         func=mybir.ActivationFunctionType.Identity,
                                         bias=bv_sb[:, 0:1], scale=1.0)
                steps.append(epi)

                def tr(sub):
                    def f():
                        if sub == 0:
                            st["vtp"] = psum.tile([128, 512], BF16, tag="acc",
                                                  bufs=3, name=f"vtp{j}")
                        nc.tensor.transpose(
                            st["vtp"][:, sub * 128:(sub + 1) * 128],
                            st["vt"][:, sub * 128:(sub + 1) * 128], ident)
                    return f

                def cp(sub):
                    def f():
                        nc.vector.tensor_copy(
                            out=v_sb[4 * j + sub],
                            in_=st["vtp"][:, sub * 128:(sub + 1) * 128])
                    return f

                for sub in range(4):
                    steps.append(tr(sub))
                steps += [cp(sub) for sub in range(4)]
            return steps

        def gen_oproj(j, tiles=range(4)):
            """Output projection for t-tiles of slice j.  Each 512-chunk:
            4 accumulating MMs + DVE copy to bf16 staging; chunked output
            DMAs on the scalar HWDGE queue."""
            steps = []
            for tt in [4 * j + s for s in tiles]:
                st = {}

                def chunk_mm(st, tt, n, h):
                    def f():
                        if n == 0 and h == 0:
                            st["ostg"] = work.tile([128, D], BF16, tag="ostg",
                                                   bufs=2, name=f"ostg{tt}")
                        if h == 0:
                            st["ops"] = psum.tile([128, 512], F32, tag="acc",
                                                  bufs=3, name=f"ops{tt}_{n}")
                        nc.tensor.matmul(
                            out=st["ops"],
                            lhsT=oT[h][:, tt * 128:(tt + 1) * 128],
                            rhs=wo_sb[:, h * D + n * 512:h * D + (n + 1) * 512],
                            start=(h == 0), stop=(h == HPC - 1))
                    return f

                def chunk_cp(st, tt, n):
                    # alternate DVE/ACT so the oproj PSUM release never
                    # queues behind the DVE racc chains
                    def f():
                        if n % 2 == 0:
                            nc.vector.tensor_copy(
                                out=st["ostg"][:, n * 512:(n + 1) * 512],
                                in_=st["ops"])
                        else:
                            nc.scalar.copy(
                                out=st["ostg"][:, n * 512:(n + 1) * 512],
                                in_=st["ops"])
                    return f

                def out_dma(st, tt, half):
                    def f():
                        nc.scalar.dma_start(
                            out=part[tt * 128:(tt + 1) * 128,
                                     half * 1024:(half + 1) * 1024],
                            in_=st["ostg"][:, half * 1024:(half + 1) * 1024])
                    return f

                for n in range(4):
                    for h in range(HPC):
                        steps.append(chunk_mm(st, tt, n, h))
                    steps.append(chunk_cp(st, tt, n))
                    if n % 2 == 1:
                        steps.append(out_dma(st, tt, n // 2))
            return steps

        # ------- attention for slice j with filler interleave -------

        def emit_attention(j, filler):
            sl = slice(j * 512, (j + 1) * 512)
            ntk = 4 * (j + 1)
            nblocks = HPC * ntk
            bdone = 0

            def pop_filler():
                nonlocal bdone
                bdone += 1
                rem_blocks = nblocks - bdone
                if not filler:
                    return
                if rem_blocks <= 0:
                    while filler:
                        filler.pop(0)()
                    return
                k = (len(filler) + rem_blocks - 1) // rem_blocks
                for _ in range(min(k, len(filler))):
                    filler.pop(0)()

            for h in range(HPC):
                st = {"sps": {}, "pt": {}}

                def score(tkb):
                    sps = psum.tile([128, 512], F32, tag="sp", bufs=3,
                                    name=f"sps{j}_{h}_{tkb}")
                    nc.tensor.matmul(out=sps,
                                     lhsT=kT[:, tkb * 128:(tkb + 1) * 128],
                                     rhs=qT[h][:, sl],
                                     start=True, stop=True)
                    pt = work.tile([128, 512], BF16, tag="pt", bufs=6,
                                   name=f"pt{j}_{h}_{tkb}")
                    nc.scalar.activation(out=pt, in_=sps,
                                         func=mybir.ActivationFunctionType.Exp,
                                         scale=SCALE)
                    if tkb >= 4 * j:
                        nc.gpsimd.affine_select(
                            out=pt, in_=pt,
                            compare_op=mybir.AluOpType.is_ge,
                            fill=0.0,
                            base=-(128 * (tkb - 4 * j)),
                            pattern=[[1, 512]],
                            channel_multiplier=-1,
                        )
                    st["pt"][tkb] = pt

                def consume(tkb):
                    pt = st["pt"].pop(tkb)
                    if tkb == 0:
                        st["otps"] = psum.tile([128, 512], F32, tag="ot",
                                               bufs=2, name=f"otps{j}_{h}")
                        st["racc"] = work.tile([128, 512], BF16, tag="racc",
                                               bufs=2, name=f"racc{j}_{h}")
                    nc.tensor.matmul(out=st["otps"], lhsT=v_sb[tkb], rhs=pt,
                                     start=(tkb == 0), stop=(tkb == ntk - 1))
                    if tkb == 0:
                        nc.vector.tensor_copy(out=st["racc"], in_=pt)
                    else:
                        nc.vector.tensor_add(out=st["racc"], in0=st["racc"],
                                             in1=pt)

                # software pipeline: score k+2 runs ahead of AV k
                for tkb in range(ntk):
                    score(tkb)
                    if tkb >= 2:
                        consume(tkb - 2)
                    pop_filler()
                for tkb in range(max(0, ntk - 2), ntk):
                    consume(tkb)

                # finalize head: rowsum via ones-matmul, normalize
                rsb = psum.tile([128, 512], F32, tag="acc", bufs=3,
                                name=f"rsb{j}_{h}")
                nc.tensor.matmul(out=rsb, lhsT=ones_bf, rhs=st["racc"],
                                 start=True, stop=True)
                rinv = work.tile([128, 512], F32, tag="rinv", bufs=2,
                                 name=f"rinv{j}_{h}")
                nc.vector.reciprocal_approx_fast(rinv, rsb)
                nc.vector.tensor_mul(out=oT[h][:, sl], in0=st["otps"],
                                     in1=rinv)

            # leftover filler (normally consumed inside the loop)
            while filler:
                filler.pop(0)()

        # ------- program -------
        # prologue: slice-0 projections, directly.  K and V first (their
        # weights land before wq), so the PE has work during the wq DMA.
        for f in (gen_proj(0, "k") + gen_proj(0, "v") + gen_proj(0, "q", 0)
                  + gen_proj(0, "q", 1) + gen_proj(0, "q", 2)
                  + gen_proj(0, "q", 3)):
            f()

        # filler distribution: defer half of oproj(1) to slice 3 so the
        # long slice-3 attention keeps enough PE filler
        for j in range(NSL):
            filler = []
            if j + 1 < NSL:
                filler += gen_proj(j + 1, "k")
                for h in range(HPC):
                    filler += gen_proj(j + 1, "q", h)
                filler += gen_proj(j + 1, "v")
            if j == 1:
                filler += gen_oproj(0)
            elif j == 2:
                filler += gen_oproj(1, tiles=(0, 1))
            elif j == 3:
                filler += gen_oproj(1, tiles=(2, 3))
                filler += gen_oproj(2)
            emit_attention(j, filler)

        # epilogue: last slice's output projection
        for f in gen_oproj(NSL - 1):
            f()

    nc.compile()
    return nc


def _get_nc():
    if "nc" not in _CACHE:
        _CACHE["nc"] = _build_nc()
    return _CACHE["nc"]


def _bf16(a):
    return np.ascontiguousarray(a.astype(ml_dtypes.bfloat16))


def kernel(x, Wq, bq, Wk, bk, Wv, bv, Wo, bo, **kw):
    x = np.asarray(x, dtype=np.float32)
    Wq = np.asarray(Wq, dtype=np.float32)
    Wk = np.asarray(Wk, dtype=np.float32)
    Wv = np.asarray(Wv, dtype=np.float32)
    Wo = np.asarray(Wo, dtype=np.float32)
    bq = np.asarray(bq, dtype=np.float32)
    bk = np.asarray(bk, dtype=np.float32)
    bv = np.asarray(bv, dtype=np.float32)
    bo = np.asarray(bo, dtype=np.float32)

    nc = _get_nc()

    # x slices, shared per batch: xs[b][j] = [128, 16*512] with columns
    # (kb, t') st. xs[b][j][p, kb*512+t'] = x[b, j*512+t', kb*128+p]
    xs_b = []
    for b in range(B):
        xT = np.ascontiguousarray(x[b].T)            # [D, T]
        xs = xT.reshape(16, 128, NSL, 512).transpose(2, 1, 0, 3)
        xs_b.append(_bf16(xs.reshape(NSL, 128, 16 * 512)))

    # per head-quarter weight packs, shared across batches
    packs = []
    for q in range(HPC):
        hs = q * HPC * DH
        kv = q // 2
        wqp = _bf16(Wq[:, hs:hs + HPC * DH].reshape(16, 128, HPC * DH)
                    .transpose(1, 0, 2).reshape(128, 16 * 512))
        wkp = _bf16(Wk[:, kv * DH:(kv + 1) * DH].reshape(16, 128, DH)
                    .transpose(1, 0, 2).reshape(128, 16 * 128))
        wvp = _bf16(Wv[:, kv * DH:(kv + 1) * DH].reshape(16, 128, DH)
                    .transpose(1, 0, 2).reshape(128, 16 * 128))
        wop = _bf16(Wo[hs:hs + HPC * DH, :].reshape(HPC, 128, D)
                    .transpose(1, 0, 2).reshape(128, HPC * D))
        bq_m = np.ascontiguousarray(
            bq[hs:hs + HPC * DH].reshape(HPC, DH).T)          # [128, 4]
        bk_m = np.ascontiguousarray(
            bk[kv * DH:(kv + 1) * DH].reshape(DH, 1))         # [128, 1]
        bv_m = np.ascontiguousarray(
            bv[kv * DH:(kv + 1) * DH].reshape(DH, 1))         # [128, 1]
        packs.append((wqp, wkp, wvp, wop, bq_m, bk_m, bv_m))

    in_maps = []
    for c in range(NCORES):
        b = c // 4
        q = c % 4
        wqp, wkp, wvp, wop, bq_m, bk_m, bv_m = packs[q]
        m = {f"xs{j}": xs_b[b][j] for j in range(NSL)}
        m.update({
            "wqx": wqp, "wkx": wkp, "wvx": wvp, "wox": wop,
            "bqm": bq_m, "bkm": bk_m, "bvm": bv_m,
        })
        in_maps.append(m)

    res = run_bass_kernel_spmd(nc, in_maps, list(range(NCORES)),
                               **kw.get("_run_kwargs", {}))
    if kw.get("_return_res"):
        return res
    parts = [res.results[c]["part"] for c in range(NCORES)]
    out = np.empty((B, T, D), dtype=np.float32)
    for b in range(B):
        acc = parts[4 * b].astype(np.float32)
        for q in range(1, 4):
            acc = acc + parts[4 * b + q].astype(np.float32)
        out[b] = acc + bo[None, :]
    return out


# revision 15
# speedup vs baseline: 1.4230x; 1.0082x over previous
"""GQA kernel for Trainium2, 8 NeuronCores.

Problem: B=2, T=2048, D=2048, 16 query heads / 2 KV heads, d_head=128, causal.

Sharding: core c -> batch b = c//4, head-quarter q = c%4 (query heads
4q..4q+3, kv head q//2). Each core computes its 4 heads' attention and a
partial output projection (its Wo rows); host sums the 4 partials per batch
and adds bo.

Host marshalling: all inputs pre-cast to bf16 and packed into [128, N]
arrays whose column layout equals the SBUF tile layout, so each logical
group is ONE large DMA with multi-KB contiguous rows, issued on a single
HWDGE queue in exact consumption order:
  wk, x-slice0, wq, wv, x-slice1, wo, x-slice2, x-slice3.

On-core dataflow (bf16 matmuls, fp32 PSUM accum), 4 rounds over 512-wide
t-slices. Per slice j the ACT-heavy attention blocks (score -> exp ->
[causal mask] -> AV) are emitted software-pipelined (score k+2 ahead of
AV k) with the PE-dense filler work (projections of slice j+1, output
projection of slice j-1) spread between them one matmul at a time, so the
statically-ordered PE queue never waits on ACT/DVE progress.

PSUM: sps bufs=3 (tag sp), otps bufs=2 (tag ot, long-lived across the tk
loop), everything else (proj accums, rowsum, oproj, V-transpose) rotates
through tag acc bufs=3.  Row sums accumulate on DVE in bf16 (2x mode);
output partials are written bf16 (host sums in fp32).
"""

import numpy as np
import ml_dtypes
from contextlib import ExitStack

import concourse.bass as bass
from concourse import bacc
import concourse.mybir as mybir
import concourse.tile as tile
from concourse.bass_utils import run_bass_kernel_spmd
from concourse.masks import make_identity

F32 = mybir.dt.float32
BF16 = mybir.dt.bfloat16

D = 2048
T = 2048
DH = 128
B = 2
HPC = 4            # query heads per core
NCORES = 8
NSL = 4            # t-slices of 512
SCALE = 1.0 / float(np.sqrt(128.0))

_CACHE = {}


def _build_nc():
    nc = bacc.Bacc("TRN2", target_bir_lowering=False, debug=False,
                   num_devices=NCORES)

    # packed inputs: one dram tensor per DMA group, rows are the SBUF
    # partition lines (multi-KB contiguous per row)
    xsd = [nc.dram_tensor(f"xs{j}", [128, 16 * 512], BF16,
                          kind="ExternalInput") for j in range(NSL)]
    wkd = nc.dram_tensor("wkx", [128, 16 * 128], BF16, kind="ExternalInput")
    wqd = nc.dram_tensor("wqx", [128, 16 * 512], BF16, kind="ExternalInput")
    wvd = nc.dram_tensor("wvx", [128, 16 * 128], BF16, kind="ExternalInput")
    wod = nc.dram_tensor("wox", [128, HPC * D], BF16, kind="ExternalInput")
    bqm = nc.dram_tensor("bqm", [DH, HPC], F32, kind="ExternalInput")
    bkm = nc.dram_tensor("bkm", [DH, 1], F32, kind="ExternalInput")
    bvm = nc.dram_tensor("bvm", [DH, 1], F32, kind="ExternalInput")
    part = nc.dram_tensor("part", [T, D], BF16, kind="ExternalOutput")

    with ExitStack() as ctx:
        tc = ctx.enter_context(tile.TileContext(nc))
        persist = ctx.enter_context(tc.tile_pool(name="persist", bufs=1))
        work = ctx.enter_context(tc.tile_pool(name="work", bufs=3))
        psum = ctx.enter_context(tc.tile_pool(name="psum", bufs=2, space="PSUM"))

        # ---- constants ----
        ones_bf = persist.tile([128, 128], BF16, tag="ones", name="ones_bf")
        nc.vector.memset(ones_bf, 1.0)
        ident = persist.tile([128, 128], BF16, tag="ident", name="ident")
        make_identity(nc, ident)

        bq_sb = persist.tile([DH, HPC], F32, tag="bq", name="bq_sb")
        nc.scalar.dma_start(out=bq_sb, in_=bqm[:, :])
        bk_sb = persist.tile([DH, 1], F32, tag="bk", name="bk_sb")
        nc.scalar.dma_start(out=bk_sb, in_=bkm[:, :])
        bv_sb = persist.tile([DH, 1], F32, tag="bv", name="bv_sb")
        nc.scalar.dma_start(out=bv_sb, in_=bvm[:, :])

        # ---- inputs -> SBUF on the sync HWDGE queue, in exact consumption
        # order.  Slice-0 groups are split fine-grained so the first
        # projection matmuls start as early as possible. ----
        xs0_sb = [persist.tile([128, 4 * 512], BF16, tag=f"xs0_{g}",
                               name=f"xs0_sb{g}") for g in range(4)]
        xs_sb = [None] + [persist.tile([128, 16 * 512], BF16, tag=f"xs{j}",
                                       name=f"xs_sb{j}") for j in range(1, NSL)]
        wk_sb = persist.tile([128, 16 * 128], BF16, tag="wk", name="wk_sb")
        wq2_sb = [persist.tile([128, 8 * 512], BF16, tag=f"wq{g}",
                               name=f"wq_sb{g}") for g in range(2)]
        wv_sb = persist.tile([128, 16 * 128], BF16, tag="wv", name="wv_sb")
        wo_sb = persist.tile([128, HPC * D], BF16, tag="wo", name="wo_sb")

        nc.sync.dma_start(out=wk_sb[:, 0:512], in_=wkd[:, 0:512])
        nc.sync.dma_start(out=wk_sb[:, 512:2048], in_=wkd[:, 512:2048])
        for g in range(4):
            nc.sync.dma_start(out=xs0_sb[g],
                              in_=xsd[0][:, g * 2048:(g + 1) * 2048])
        nc.sync.dma_start(out=wv_sb, in_=wvd[:, :])
        for g in range(2):
            nc.sync.dma_start(out=wq2_sb[g],
                              in_=wqd[:, g * 4096:(g + 1) * 4096])
        nc.sync.dma_start(out=xs_sb[1], in_=xsd[1][:, :])
        nc.sync.dma_start(out=wo_sb, in_=wod[:, :])
        nc.sync.dma_start(out=xs_sb[2], in_=xsd[2][:, :])
        nc.sync.dma_start(out=xs_sb[3], in_=xsd[3][:, :])

        def xrhs(j, kb):
            if j == 0:
                return xs0_sb[kb // 4][:, (kb % 4) * 512:(kb % 4 + 1) * 512]
            return xs_sb[j][:, kb * 512:(kb + 1) * 512]

        def wqap(kb, h):
            return wq2_sb[kb // 8][:, (kb % 8) * 512 + h * 128:
                                   (kb % 8) * 512 + (h + 1) * 128]

        # ---- persistent activations ----
        qT = [persist.tile([128, T], BF16, tag=f"qT{h}", name=f"qT{h}")
              for h in range(HPC)]
        kT = persist.tile([128, T], BF16, tag="kT", name="kT")
        v_sb = [persist.tile([128, DH], BF16, tag=f"v{t}", name=f"v{t}")
                for t in range(16)]
        oT = [persist.tile([128, T], BF16, tag=f"oT{h}", name=f"oT{h}")
              for h in range(HPC)]

        # ------- emission helpers; each returns a list of closures that
        # emit ONE instruction each (plus tile allocs), used as PE filler
        # interleaved into the attention stream -------

        def gen_proj(j, kind, h=0):
            """Projection group for slice j: 16 accumulating MMs + ACT
            epilogue. kind in {'k','q','v'}."""
            sl = slice(j * 512, (j + 1) * 512)
            st = {}

            def mm(kb):
                def f():
                    if kb == 0:
                        st["ps"] = psum.tile([128, 512], F32, tag="acc",
                                             bufs=3, name=f"{kind}ps{j}_{h}")
                    if kind == "k":
                        lhsT = wk_sb[:, kb * 128:(kb + 1) * 128]
                    elif kind == "v":
                        lhsT = wv_sb[:, kb * 128:(kb + 1) * 128]
                    else:
                        lhsT = wqap(kb, h)
                    nc.tensor.matmul(out=st["ps"], lhsT=lhsT,
                                     rhs=xrhs(j, kb),
                                     start=(kb == 0), stop=(kb == 15))
                return f

            steps = [mm(kb) for kb in range(16)]

            if kind == "k":
                def epi():
                    nc.scalar.activation(out=kT[:, sl], in_=st["ps"],
                                         func=mybir.ActivationFunctionType.Identity,
                                         bias=bk_sb[:, 0:1], scale=1.0)
                steps.append(epi)
            elif kind == "q":
                def epi():
                    nc.scalar.activation(out=qT[h][:, sl], in_=st["ps"],
                                         func=mybir.ActivationFunctionType.Identity,
                                         bias=bq_sb[:, h:h + 1], scale=1.0)
                steps.append(epi)
            else:
                def epi():
                    st["vt"] = work.tile([128, 512], BF16, tag="vt", bufs=2,
                                         name=f"vt{j}")
                    nc.scalar.activation(out=st["vt"], in_=st["ps"],
                                         func=mybir.ActivationFunctionType.Identity,
                                         bias=bv_sb[:, 0:1], scale=1.0)
                steps.append(epi)

                def tr(sub):
                    def f():
                        if sub == 0:
                            st["vtp"] = psum.tile([128, 512], BF16, tag="acc",
                                                  bufs=3, name=f"vtp{j}")
                        nc.tensor.transpose(
                            st["vtp"][:, sub * 128:(sub + 1) * 128],
                            st["vt"][:, sub * 128:(sub + 1) * 128], ident)
                    return f

                def cp(sub):
                    def f():
                        nc.vector.tensor_copy(
                            out=v_sb[4 * j + sub],
                            in_=st["vtp"][:, sub * 128:(sub + 1) * 128])
                    return f

                for sub in range(4):
                    steps.append(tr(sub))
                steps += [cp(sub) for sub in range(4)]
            return steps

        def gen_oproj(j, tiles=range(4)):
            """Output projection for t-tiles of slice j.  Each 512-chunk:
            4 accumulating MMs + DVE copy to bf16 staging; chunked output
            DMAs on the scalar HWDGE queue."""
            steps = []
            for tt in [4 * j + s for s in tiles]:
                st = {}

                def chunk_mm(st, tt, n, h):
                    def f():
                        if n == 0 and h == 0:
                            st["ostg"] = work.tile([128, D], BF16, tag="ostg",
                                                   bufs=2, name=f"ostg{tt}")
                        if h == 0:
                            st["ops"] = psum.tile([128, 512], F32, tag="acc",
                                                  bufs=3, name=f"ops{tt}_{n}")
                        nc.tensor.matmul(
                            out=st["ops"],
                            lhsT=oT[h][:, tt * 128:(tt + 1) * 128],
                            rhs=wo_sb[:, h * D + n * 512:h * D + (n + 1) * 512],
                            start=(h == 0), stop=(h == HPC - 1))
                    return f

                def chunk_cp(st, tt, n):
                    # alternate DVE/ACT so the oproj PSUM release never
                    # queues behind the DVE racc chains
                    def f():
                        if n % 2 == 0:
                            nc.vector.tensor_copy(
                                out=st["ostg"][:, n * 512:(n + 1) * 512],
                                in_=st["ops"])
                        else:
                            nc.scalar.copy(
                                out=st["ostg"][:, n * 512:(n + 1) * 512],
                                in_=st["ops"])
                    return f

                def out_dma(st, tt, half):
                    def f():
                        nc.scalar.dma_start(
                            out=part[tt * 128:(tt + 1) * 128,
                                     half * 1024:(half + 1) * 1024],
                            in_=st["ostg"][:, half * 1024:(half + 1) * 1024])
                    return f

                for n in range(4):
                    for h in range(HPC):
                        steps.append(chunk_mm(st, tt, n, h))
                    steps.append(chunk_cp(st, tt, n))
                    if n % 2 == 1:
                        steps.append(out_dma(st, tt, n // 2))
            return steps

        # ------- attention for slice j with filler interleave -------

        def emit_attention(j, filler):
            sl = slice(j * 512, (j + 1) * 512)
            ntk = 4 * (j + 1)
            nblocks = HPC * ntk
            bdone = 0

            def pop_filler():
                nonlocal bdone
                bdone += 1
                rem_blocks = nblocks - bdone
                if not filler:
                    return
                if rem_blocks <= 0:
                    while filler:
                        filler.pop(0)()
                    return
                k = (len(filler) + rem_blocks - 1) // rem_blocks
                for _ in range(min(k, len(filler))):
                    filler.pop(0)()

            fin_prev = [None]
            for h in range(HPC):
                st = {"sps": {}, "pt": {}}

                def score(tkb):
                    sps = psum.tile([128, 512], F32, tag="sp", bufs=3,
                                    name=f"sps{j}_{h}_{tkb}")
                    nc.tensor.matmul(out=sps,
                                     lhsT=kT[:, tkb * 128:(tkb + 1) * 128],
                                     rhs=qT[h][:, sl],
                                     start=True, stop=True)
                    pt = work.tile([128, 512], BF16, tag="pt", bufs=6,
                                   name=f"pt{j}_{h}_{tkb}")
                    nc.scalar.activation(out=pt, in_=sps,
                                         func=mybir.ActivationFunctionType.Exp,
                                         scale=SCALE)
                    if tkb >= 4 * j:
                        nc.gpsimd.affine_select(
                            out=pt, in_=pt,
                            compare_op=mybir.AluOpType.is_ge,
                            fill=0.0,
                            base=-(128 * (tkb - 4 * j)),
                            pattern=[[1, 512]],
                            channel_multiplier=-1,
                        )
                    st["pt"][tkb] = pt

                def consume(tkb):
                    pt = st["pt"].pop(tkb)
                    if tkb == 0:
                        st["otps"] = psum.tile([128, 512], F32, tag="ot",
                                               bufs=2, name=f"otps{j}_{h}")
                        st["racc"] = work.tile([128, 512], BF16, tag="racc",
                                               bufs=2, name=f"racc{j}_{h}")
                    nc.tensor.matmul(out=st["otps"], lhsT=v_sb[tkb], rhs=pt,
                                     start=(tkb == 0), stop=(tkb == ntk - 1))
                    if tkb == 0:
                        nc.vector.tensor_copy(out=st["racc"], in_=pt)
                    else:
                        nc.vector.tensor_add(out=st["racc"], in0=st["racc"],
                                             in1=pt)

                def make_fin(h, st):
                    # finalize head: rowsum via ones-matmul, normalize.
                    # Deferred into the NEXT head's stream so the rowsum MM
                    # never heads the PE queue while the DVE racc chain is
                    # still draining.
                    def fin():
                        rsb = psum.tile([128, 512], F32, tag="acc", bufs=3,
                                        name=f"rsb{j}_{h}")
                        nc.tensor.matmul(out=rsb, lhsT=ones_bf,
                                         rhs=st["racc"],
                                         start=True, stop=True)
                        rinv = work.tile([128, 512], F32, tag="rinv", bufs=2,
                                         name=f"rinv{j}_{h}")
                        nc.vector.reciprocal_approx_fast(rinv, rsb)
                        nc.vector.tensor_mul(out=oT[h][:, sl],
                                             in0=st["otps"], in1=rinv)
                    return fin

                # software pipeline: score k+2 runs ahead of AV k
                for tkb in range(ntk):
                    score(tkb)
                    if tkb == 1 and fin_prev[0] is not None:
                        fin_prev[0]()
                        fin_prev[0] = None
                    if tkb >= 2:
                        consume(tkb - 2)
                    pop_filler()
                for tkb in range(max(0, ntk - 2), ntk):
                    consume(tkb)
                fin_prev[0] = make_fin(h, st)

            # last head's FIN goes behind a bit of leftover filler (filler
            # never reads this slice's oT, so this is order-safe)
            for _ in range(min(6, len(filler))):
                filler.pop(0)()
            fin_prev[0]()
            fin_prev[0] = None
            # leftover filler (normally consumed inside the loop)
            while filler:
                filler.pop(0)()

        # ------- program -------
        # prologue: slice-0 projections, directly.  K and V first (their
        # weights land before wq), so the PE has work during the wq DMA.
        for f in (gen_proj(0, "k") + gen_proj(0, "v") + gen_proj(0, "q", 0)
                  + gen_proj(0, "q", 1) + gen_proj(0, "q", 2)
                  + gen_proj(0, "q", 3)):
            f()

        # filler distribution: defer half of oproj(1) to slice 3 so the
        # long slice-3 attention keeps enough PE filler
        for j in range(NSL):
            filler = []
            if j + 1 < NSL:
                filler += gen_proj(j + 1, "k")
                for h in range(HPC):
                    filler += gen_proj(j + 1, "q", h)
                filler += gen_proj(j + 1, "v")
            if j == 1:
                filler += gen_oproj(0)
            elif j == 2:
                filler += gen_oproj(1, tiles=(0, 1))
            elif j == 3:
                filler += gen_oproj(1, tiles=(2, 3))
                filler += gen_oproj(2)
            emit_attention(j, filler)

        # epilogue: last slice's output projection
        for f in gen_oproj(NSL - 1):
            f()

    nc.compile()
    return nc


def _get_nc():
    if "nc" not in _CACHE:
        _CACHE["nc"] = _build_nc()
    return _CACHE["nc"]


def _bf16(a):
    return np.ascontiguousarray(a.astype(ml_dtypes.bfloat16))


def kernel(x, Wq, bq, Wk, bk, Wv, bv, Wo, bo, **kw):
    x = np.asarray(x, dtype=np.float32)
    Wq = np.asarray(Wq, dtype=np.float32)
    Wk = np.asarray(Wk, dtype=np.float32)
    Wv = np.asarray(Wv, dtype=np.float32)
    Wo = np.asarray(Wo, dtype=np.float32)
    bq = np.asarray(bq, dtype=np.float32)
    bk = np.asarray(bk, dtype=np.float32)
    bv = np.asarray(bv, dtype=np.float32)
    bo = np.asarray(bo, dtype=np.float32)

    nc = _get_nc()

    # x slices, shared per batch: xs[b][j] = [128, 16*512] with columns
    # (kb, t') st. xs[b][j][p, kb*512+t'] = x[b, j*512+t', kb*128+p]
    xs_b = []
    for b in range(B):
        xT = np.ascontiguousarray(x[b].T)            # [D, T]
        xs = xT.reshape(16, 128, NSL, 512).transpose(2, 1, 0, 3)
        xs_b.append(_bf16(xs.reshape(NSL, 128, 16 * 512)))

    # per head-quarter weight packs, shared across batches
    packs = []
    for q in range(HPC):
        hs = q * HPC * DH
        kv = q // 2
        wqp = _bf16(Wq[:, hs:hs + HPC * DH].reshape(16, 128, HPC * DH)
                    .transpose(1, 0, 2).reshape(128, 16 * 512))
        wkp = _bf16(Wk[:, kv * DH:(kv + 1) * DH].reshape(16, 128, DH)
                    .transpose(1, 0, 2).reshape(128, 16 * 128))
        wvp = _bf16(Wv[:, kv * DH:(kv + 1) * DH].reshape(16, 128, DH)
                    .transpose(1, 0, 2).reshape(128, 16 * 128))
        wop = _bf16(Wo[hs:hs + HPC * DH, :].reshape(HPC, 128, D)
                    .transpose(1, 0, 2).reshape(128, HPC * D))
        bq_m = np.ascontiguousarray(
            bq[hs:hs + HPC * DH].reshape(HPC, DH).T)          # [128, 4]
        bk_m = np.ascontiguousarray(
            bk[kv * DH:(kv + 1) * DH].reshape(DH, 1))         # [128, 1]
        bv_m = np.ascontiguousarray(
            bv[kv * DH:(kv + 1) * DH].reshape(DH, 1))         # [128, 1]
        packs.append((wqp, wkp, wvp, wop, bq_m, bk_m, bv_m))

    in_maps = []
    for c in range(NCORES):
        b = c // 4
        q = c % 4
        wqp, wkp, wvp, wop, bq_m, bk_m, bv_m = packs[q]
        m = {f"xs{j}": xs_b[b][j] for j in range(NSL)}
        m.update({
            "wqx": wqp, "wkx": wkp, "wvx": wvp, "wox": wop,
            "bqm": bq_m, "bkm": bk_m, "bvm": bv_m,
        })
        in_maps.append(m)

    res = run_bass_kernel_spmd(nc, in_maps, list(range(NCORES)),
                               **kw.get("_run_kwargs", {}))
    if kw.get("_return_res"):
        return res
    parts = [res.results[c]["part"] for c in range(NCORES)]
    out = np.empty((B, T, D), dtype=np.float32)
    for b in range(B):
        acc = parts[4 * b].astype(np.float32)
        for q in range(1, 4):
            acc = acc + parts[4 * b + q].astype(np.float32)
        out[b] = acc + bo[None, :]
    return out


# revision 17
# speedup vs baseline: 1.4368x; 1.0097x over previous
"""GQA kernel for Trainium2, 8 NeuronCores.

Problem: B=2, T=2048, D=2048, 16 query heads / 2 KV heads, d_head=128, causal.

Sharding: core c -> batch b = c//4, head-quarter q = c%4 (query heads
4q..4q+3, kv head q//2). Each core computes its 4 heads' attention and a
partial output projection (its Wo rows); host sums the 4 partials per batch
and adds bo.

Host marshalling: all inputs pre-cast to bf16 and packed into [128, N]
arrays whose column layout equals the SBUF tile layout, so each logical
group is ONE large DMA with multi-KB contiguous rows, issued on a single
HWDGE queue in exact consumption order:
  wk, x-slice0, wq, wv, x-slice1, wo, x-slice2, x-slice3.

On-core dataflow (bf16 matmuls, fp32 PSUM accum), 4 rounds over 512-wide
t-slices. Per slice j the ACT-heavy attention blocks (score -> exp ->
[causal mask] -> AV) are emitted software-pipelined (score k+2 ahead of
AV k) with the PE-dense filler work (projections of slice j+1, output
projection of slice j-1) spread between them one matmul at a time, so the
statically-ordered PE queue never waits on ACT/DVE progress.

PSUM: sps bufs=3 (tag sp), otps bufs=2 (tag ot, long-lived across the tk
loop), everything else (proj accums, rowsum, oproj, V-transpose) rotates
through tag acc bufs=3.  Row sums accumulate on DVE in bf16 (2x mode);
output partials are written bf16 (host sums in fp32).
"""

import numpy as np
import ml_dtypes
from contextlib import ExitStack

import concourse.bass as bass
from concourse import bacc
import concourse.mybir as mybir
import concourse.tile as tile
from concourse.bass_utils import run_bass_kernel_spmd
from concourse.masks import make_identity

F32 = mybir.dt.float32
BF16 = mybir.dt.bfloat16

D = 2048
T = 2048
DH = 128
B = 2
HPC = 4            # query heads per core
NCORES = 8
NSL = 4            # t-slices of 512
SCALE = 1.0 / float(np.sqrt(128.0))

_CACHE = {}


def _build_nc():
    nc = bacc.Bacc("TRN2", target_bir_lowering=False, debug=False,
                   num_devices=NCORES)

    # packed inputs: one dram tensor per DMA group, rows are the SBUF
    # partition lines (multi-KB contiguous per row)
    xsd = [nc.dram_tensor(f"xs{j}", [128, 16 * 512], BF16,
                          kind="ExternalInput") for j in range(NSL)]
    wkd = nc.dram_tensor("wkx", [128, 16 * 128], BF16, kind="ExternalInput")
    wqd = nc.dram_tensor("wqx", [128, 16 * 512], BF16, kind="ExternalInput")
    wvd = nc.dram_tensor("wvx", [128, 16 * 128], BF16, kind="ExternalInput")
    wod = nc.dram_tensor("wox", [128, HPC * D], BF16, kind="ExternalInput")
    bqm = nc.dram_tensor("bqm", [DH, HPC], F32, kind="ExternalInput")
    bkm = nc.dram_tensor("bkm", [DH, 1], F32, kind="ExternalInput")
    bvm = nc.dram_tensor("bvm", [DH, 1], F32, kind="ExternalInput")
    part = nc.dram_tensor("part", [T, D], BF16, kind="ExternalOutput")

    with ExitStack() as ctx:
        tc = ctx.enter_context(tile.TileContext(nc))
        persist = ctx.enter_context(tc.tile_pool(name="persist", bufs=1))
        work = ctx.enter_context(tc.tile_pool(name="work", bufs=3))
        psum = ctx.enter_context(tc.tile_pool(name="psum", bufs=2, space="PSUM"))

        # ---- constants ----
        ones_bf = persist.tile([128, 128], BF16, tag="ones", name="ones_bf")
        nc.vector.memset(ones_bf, 1.0)
        ident = persist.tile([128, 128], BF16, tag="ident", name="ident")
        make_identity(nc, ident)

        bq_sb = persist.tile([DH, HPC], F32, tag="bq", name="bq_sb")
        nc.scalar.dma_start(out=bq_sb, in_=bqm[:, :])
        bk_sb = persist.tile([DH, 1], F32, tag="bk", name="bk_sb")
        nc.scalar.dma_start(out=bk_sb, in_=bkm[:, :])
        bv_sb = persist.tile([DH, 1], F32, tag="bv", name="bv_sb")
        nc.scalar.dma_start(out=bv_sb, in_=bvm[:, :])

        # ---- inputs -> SBUF on the sync HWDGE queue, in exact consumption
        # order.  Slice-0 groups are split fine-grained so the first
        # projection matmuls start as early as possible. ----
        xs0_sb = [persist.tile([128, 4 * 512], BF16, tag=f"xs0_{g}",
                               name=f"xs0_sb{g}") for g in range(4)]
        xs_sb = [None] + [persist.tile([128, 16 * 512], BF16, tag=f"xs{j}",
                                       name=f"xs_sb{j}") for j in range(1, NSL)]
        wk_sb = persist.tile([128, 16 * 128], BF16, tag="wk", name="wk_sb")
        wq2_sb = [persist.tile([128, 8 * 512], BF16, tag=f"wq{g}",
                               name=f"wq_sb{g}") for g in range(2)]
        wv_sb = persist.tile([128, 16 * 128], BF16, tag="wv", name="wv_sb")
        wo_sb = persist.tile([128, HPC * D], BF16, tag="wo", name="wo_sb")

        nc.sync.dma_start(out=wk_sb[:, 0:512], in_=wkd[:, 0:512])
        nc.sync.dma_start(out=wk_sb[:, 512:2048], in_=wkd[:, 512:2048])
        for g in range(4):
            nc.sync.dma_start(out=xs0_sb[g],
                              in_=xsd[0][:, g * 2048:(g + 1) * 2048])
        nc.sync.dma_start(out=wv_sb, in_=wvd[:, :])
        for g in range(2):
            nc.sync.dma_start(out=wq2_sb[g],
                              in_=wqd[:, g * 4096:(g + 1) * 4096])
        nc.sync.dma_start(out=xs_sb[1], in_=xsd[1][:, :])
        nc.sync.dma_start(out=wo_sb, in_=wod[:, :])
        nc.sync.dma_start(out=xs_sb[2], in_=xsd[2][:, :])
        nc.sync.dma_start(out=xs_sb[3], in_=xsd[3][:, :])

        def xrhs(j, kb):
            if j == 0:
                return xs0_sb[kb // 4][:, (kb % 4) * 512:(kb % 4 + 1) * 512]
            return xs_sb[j][:, kb * 512:(kb + 1) * 512]

        def wqap(kb, h):
            return wq2_sb[kb // 8][:, (kb % 8) * 512 + h * 128:
                                   (kb % 8) * 512 + (h + 1) * 128]

        # ---- persistent activations ----
        qT = [persist.tile([128, T], BF16, tag=f"qT{h}", name=f"qT{h}")
              for h in range(HPC)]
        kT = persist.tile([128, T], BF16, tag="kT", name="kT")
        v_sb = [persist.tile([128, DH], BF16, tag=f"v{t}", name=f"v{t}")
                for t in range(16)]
        oT = [persist.tile([128, T], BF16, tag=f"oT{h}", name=f"oT{h}")
              for h in range(HPC)]

        # ------- emission helpers; each returns a list of closures that
        # emit ONE instruction each (plus tile allocs), used as PE filler
        # interleaved into the attention stream -------

        def gen_proj(j, kind, h=0):
            """Projection group for slice j: 16 accumulating MMs + ACT
            epilogue. kind in {'k','q','v'}."""
            sl = slice(j * 512, (j + 1) * 512)
            st = {}

            def mm(kb):
                def f():
                    if kb == 0:
                        st["ps"] = psum.tile([128, 512], F32, tag="acc",
                                             bufs=3, name=f"{kind}ps{j}_{h}")
                    if kind == "k":
                        lhsT = wk_sb[:, kb * 128:(kb + 1) * 128]
                    elif kind == "v":
                        lhsT = wv_sb[:, kb * 128:(kb + 1) * 128]
                    else:
                        lhsT = wqap(kb, h)
                    nc.tensor.matmul(out=st["ps"], lhsT=lhsT,
                                     rhs=xrhs(j, kb),
                                     start=(kb == 0), stop=(kb == 15))
                return f

            steps = [mm(kb) for kb in range(16)]

            if kind == "k":
                def epi():
                    nc.scalar.activation(out=kT[:, sl], in_=st["ps"],
                                         func=mybir.ActivationFunctionType.Identity,
                                         bias=bk_sb[:, 0:1], scale=1.0)
                steps.append(epi)
            elif kind == "q":
                def epi():
                    nc.scalar.activation(out=qT[h][:, sl], in_=st["ps"],
                                         func=mybir.ActivationFunctionType.Identity,
                                         bias=bq_sb[:, h:h + 1], scale=1.0)
                steps.append(epi)
            else:
                def epi():
                    st["vt"] = work.tile([128, 512], BF16, tag="vt", bufs=2,
                                         name=f"vt{j}")
                    nc.scalar.activation(out=st["vt"], in_=st["ps"],
                                         func=mybir.ActivationFunctionType.Identity,
                                         bias=bv_sb[:, 0:1], scale=1.0)
                steps.append(epi)

                def tr(sub):
                    def f():
                        if sub == 0:
                            st["vtp"] = psum.tile([128, 512], BF16, tag="acc",
                                                  bufs=3, name=f"vtp{j}")
                        nc.tensor.transpose(
                            st["vtp"][:, sub * 128:(sub + 1) * 128],
                            st["vt"][:, sub * 128:(sub + 1) * 128], ident)
                    return f

                def cp(sub):
                    def f():
                        nc.vector.tensor_copy(
                            out=v_sb[4 * j + sub],
                            in_=st["vtp"][:, sub * 128:(sub + 1) * 128])
                    return f

                for sub in range(4):
                    steps.append(tr(sub))
                steps += [cp(sub) for sub in range(4)]
            return steps

        def gen_oproj(j, tiles=range(4)):
            """Output projection for t-tiles of slice j.  Each 512-chunk:
            4 accumulating MMs + DVE copy to bf16 staging; chunked output
            DMAs on the scalar HWDGE queue."""
            steps = []
            for tt in [4 * j + s for s in tiles]:
                st = {}

                def chunk_mm(st, tt, n, h):
                    def f():
                        if n == 0 and h == 0:
                            st["ostg"] = work.tile([128, D], BF16, tag="ostg",
                                                   bufs=2, name=f"ostg{tt}")
                        if h == 0:
                            st["ops"] = psum.tile([128, 512], F32, tag="acc",
                                                  bufs=3, name=f"ops{tt}_{n}")
                        nc.tensor.matmul(
                            out=st["ops"],
                            lhsT=oT[h][:, tt * 128:(tt + 1) * 128],
                            rhs=wo_sb[:, h * D + n * 512:h * D + (n + 1) * 512],
                            start=(h == 0), stop=(h == HPC - 1))
                    return f

                def chunk_cp(st, tt, n):
                    # alternate DVE/ACT so the oproj PSUM release never
                    # queues behind the DVE racc chains
                    def f():
                        if n % 2 == 0:
                            nc.vector.tensor_copy(
                                out=st["ostg"][:, n * 512:(n + 1) * 512],
                                in_=st["ops"])
                        else:
                            nc.scalar.copy(
                                out=st["ostg"][:, n * 512:(n + 1) * 512],
                                in_=st["ops"])
                    return f

                def out_dma(st, tt, half):
                    def f():
                        nc.scalar.dma_start(
                            out=part[tt * 128:(tt + 1) * 128,
                                     half * 1024:(half + 1) * 1024],
                            in_=st["ostg"][:, half * 1024:(half + 1) * 1024])
                    return f

                for n in range(4):
                    for h in range(HPC):
                        steps.append(chunk_mm(st, tt, n, h))
                    steps.append(chunk_cp(st, tt, n))
                    if n % 2 == 1:
                        steps.append(out_dma(st, tt, n // 2))
            return steps

        # ------- attention for slice j with filler interleave -------

        def emit_attention(j, filler):
            sl = slice(j * 512, (j + 1) * 512)
            ntk = 4 * (j + 1)
            nblocks = HPC * ntk
            bdone = 0

            def pop_filler():
                nonlocal bdone
                bdone += 1
                rem_blocks = nblocks - bdone
                if not filler:
                    return
                if rem_blocks <= 0:
                    while filler:
                        filler.pop(0)()
                    return
                k = (len(filler) + rem_blocks - 1) // rem_blocks
                for _ in range(min(k, len(filler))):
                    filler.pop(0)()

            fin_prev = [None]
            for h in range(HPC):
                st = {"sps": {}, "pt": {}}

                def score(tkb):
                    sps = psum.tile([128, 512], F32, tag="sp", bufs=3,
                                    name=f"sps{j}_{h}_{tkb}")
                    nc.tensor.matmul(out=sps,
                                     lhsT=kT[:, tkb * 128:(tkb + 1) * 128],
                                     rhs=qT[h][:, sl],
                                     start=True, stop=True)
                    pt = work.tile([128, 512], BF16, tag="pt", bufs=6,
                                   name=f"pt{j}_{h}_{tkb}")
                    nc.scalar.activation(out=pt, in_=sps,
                                         func=mybir.ActivationFunctionType.Exp,
                                         scale=SCALE)
                    if tkb >= 4 * j:
                        nc.gpsimd.affine_select(
                            out=pt, in_=pt,
                            compare_op=mybir.AluOpType.is_ge,
                            fill=0.0,
                            base=-(128 * (tkb - 4 * j)),
                            pattern=[[1, 512]],
                            channel_multiplier=-1,
                        )
                    st["pt"][tkb] = pt

                def consume(tkb):
                    pt = st["pt"].pop(tkb)
                    if tkb == 0:
                        st["otps"] = psum.tile([128, 512], F32, tag="ot",
                                               bufs=2, name=f"otps{j}_{h}")
                        st["racc"] = work.tile([128, 512], BF16, tag="racc",
                                               bufs=2, name=f"racc{j}_{h}")
                    nc.tensor.matmul(out=st["otps"], lhsT=v_sb[tkb], rhs=pt,
                                     start=(tkb == 0), stop=(tkb == ntk - 1))
                    if tkb == 0:
                        nc.vector.tensor_copy(out=st["racc"], in_=pt)
                    else:
                        nc.vector.tensor_add(out=st["racc"], in0=st["racc"],
                                             in1=pt)

                def make_fin(h, st):
                    # finalize head: rowsum via ones-matmul, normalize.
                    # Deferred into the NEXT head's stream so the rowsum MM
                    # never heads the PE queue while the DVE racc chain is
                    # still draining.
                    def fin():
                        rsb = psum.tile([128, 512], F32, tag="acc", bufs=3,
                                        name=f"rsb{j}_{h}")
                        nc.tensor.matmul(out=rsb, lhsT=ones_bf,
                                         rhs=st["racc"],
                                         start=True, stop=True)
                        rinv = work.tile([128, 512], F32, tag="rinv", bufs=2,
                                         name=f"rinv{j}_{h}")
                        nc.vector.reciprocal_approx_fast(rinv, rsb)
                        nc.vector.tensor_mul(out=oT[h][:, sl],
                                             in0=st["otps"], in1=rinv)
                    return fin

                # software pipeline: score k+3 runs ahead of AV k (covers
                # the exp + causal-mask latency of diagonal blocks)
                depth = min(3, ntk - 1)
                for tkb in range(ntk):
                    score(tkb)
                    if tkb == 1 and fin_prev[0] is not None:
                        fin_prev[0]()
                        fin_prev[0] = None
                    if tkb >= depth:
                        consume(tkb - depth)
                    pop_filler()
                for tkb in range(max(0, ntk - depth), ntk):
                    consume(tkb)
                fin_prev[0] = make_fin(h, st)

            # last head's FIN goes behind a bit of leftover filler (filler
            # never reads this slice's oT, so this is order-safe)
            for _ in range(min(6, len(filler))):
                filler.pop(0)()
            fin_prev[0]()
            fin_prev[0] = None
            # leftover filler (normally consumed inside the loop)
            while filler:
                filler.pop(0)()

        # ------- program -------
        # prologue: slice-0 projections, directly.  K and V first (their
        # weights land before wq), so the PE has work during the wq DMA.
        for f in (gen_proj(0, "k") + gen_proj(0, "v") + gen_proj(0, "q", 0)
                  + gen_proj(0, "q", 1) + gen_proj(0, "q", 2)
                  + gen_proj(0, "q", 3)):
            f()

        # filler distribution: defer half of oproj(1) to slice 3 so the
        # long slice-3 attention keeps enough PE filler
        for j in range(NSL):
            filler = []
            if j + 1 < NSL:
                filler += gen_proj(j + 1, "k")
                for h in range(HPC):
                    filler += gen_proj(j + 1, "q", h)
                filler += gen_proj(j + 1, "v")
            if j == 1:
                filler += gen_oproj(0)
            elif j == 3:
                filler += gen_oproj(1)
                filler += gen_oproj(2)
            emit_attention(j, filler)

        # epilogue: last slice's output projection
        for f in gen_oproj(NSL - 1):
            f()

    nc.compile()
    return nc


def _get_nc():
    if "nc" not in _CACHE:
        _CACHE["nc"] = _build_nc()
    return _CACHE["nc"]


def _bf16(a):
    return np.ascontiguousarray(a.astype(ml_dtypes.bfloat16))


def kernel(x, Wq, bq, Wk, bk, Wv, bv, Wo, bo, **kw):
    x = np.asarray(x, dtype=np.float32)
    Wq = np.asarray(Wq, dtype=np.float32)
    Wk = np.asarray(Wk, dtype=np.float32)
    Wv = np.asarray(Wv, dtype=np.float32)
    Wo = np.asarray(Wo, dtype=np.float32)
    bq = np.asarray(bq, dtype=np.float32)
    bk = np.asarray(bk, dtype=np.float32)
    bv = np.asarray(bv, dtype=np.float32)
    bo = np.asarray(bo, dtype=np.float32)

    nc = _get_nc()

    # x slices, shared per batch: xs[b][j] = [128, 16*512] with columns
    # (kb, t') st. xs[b][j][p, kb*512+t'] = x[b, j*512+t', kb*128+p]
    xs_b = []
    for b in range(B):
        xT = np.ascontiguousarray(x[b].T)            # [D, T]
        xs = xT.reshape(16, 128, NSL, 512).transpose(2, 1, 0, 3)
        xs_b.append(_bf16(xs.reshape(NSL, 128, 16 * 512)))

    # per head-quarter weight packs, shared across batches
    packs = []
    for q in range(HPC):
        hs = q * HPC * DH
        kv = q // 2
        wqp = _bf16(Wq[:, hs:hs + HPC * DH].reshape(16, 128, HPC * DH)
                    .transpose(1, 0, 2).reshape(128, 16 * 512))
        wkp = _bf16(Wk[:, kv * DH:(kv + 1) * DH].reshape(16, 128, DH)
                    .transpose(1, 0, 2).reshape(128, 16 * 128))
        wvp = _bf16(Wv[:, kv * DH:(kv + 1) * DH].reshape(16, 128, DH)
                    .transpose(1, 0, 2).reshape(128, 16 * 128))
        wop = _bf16(Wo[hs:hs + HPC * DH, :].reshape(HPC, 128, D)
                    .transpose(1, 0, 2).reshape(128, HPC * D))
        bq_m = np.ascontiguousarray(
            bq[hs:hs + HPC * DH].reshape(HPC, DH).T)          # [128, 4]
        bk_m = np.ascontiguousarray(
            bk[kv * DH:(kv + 1) * DH].reshape(DH, 1))         # [128, 1]
        bv_m = np.ascontiguousarray(
            bv[kv * DH:(kv + 1) * DH].reshape(DH, 1))         # [128, 1]
        packs.append((wqp, wkp, wvp, wop, bq_m, bk_m, bv_m))

    in_maps = []
    for c in range(NCORES):
        b = c // 4
        q = c % 4
        wqp, wkp, wvp, wop, bq_m, bk_m, bv_m = packs[q]
        m = {f"xs{j}": xs_b[b][j] for j in range(NSL)}
        m.update({
            "wqx": wqp, "wkx": wkp, "wvx": wvp, "wox": wop,
            "bqm": bq_m, "bkm": bk_m, "bvm": bv_m,
        })
        in_maps.append(m)

    res = run_bass_kernel_spmd(nc, in_maps, list(range(NCORES)),
                               **kw.get("_run_kwargs", {}))
    if kw.get("_return_res"):
        return res
    parts = [res.results[c]["part"] for c in range(NCORES)]
    out = np.empty((B, T, D), dtype=np.float32)
    for b in range(B):
        acc = parts[4 * b].astype(np.float32)
        for q in range(1, 4):
            acc = acc + parts[4 * b + q].astype(np.float32)
        out[b] = acc + bo[None, :]
    return out


# revision 18
# speedup vs baseline: 1.4469x; 1.0070x over previous
"""GQA kernel for Trainium2, 8 NeuronCores.

Problem: B=2, T=2048, D=2048, 16 query heads / 2 KV heads, d_head=128, causal.

Sharding: core c -> batch b = c//4, head-quarter q = c%4 (query heads
4q..4q+3, kv head q//2). Each core computes its 4 heads' attention and a
partial output projection (its Wo rows); host sums the 4 partials per batch
and adds bo.

Host marshalling: all inputs pre-cast to bf16 and packed into [128, N]
arrays whose column layout equals the SBUF tile layout, so each logical
group is one large DMA with multi-KB contiguous rows, issued on a single
HWDGE queue in exact consumption order:
  wk, x-slice0 (4 chunks), wv, wq (2 chunks), x-slice1, wo, x-s2, x-s3.

On-core dataflow (bf16 matmuls, fp32 PSUM accum), 4 rounds over 512-wide
t-slices. Per slice j the ACT-heavy attention blocks (score -> exp ->
[causal mask] -> AV) are emitted software-pipelined (score k+3 ahead of
AV k, covering exp + mask latency) with the PE-dense filler work
(projections of slice j+1, output projection of an earlier slice) spread
between them a few matmuls at a time, so the statically-ordered PE queue
never waits on ACT/DVE progress.  Each head's finalize (rowsum matmul,
reciprocal, normalize) is deferred into the next head's stream so it
never heads the PE queue while the DVE row-sum chain drains.

PSUM: sps bufs=3 (tag sp), otps bufs=2 (tag ot, long-lived across the tk
loop), everything else (proj accums, rowsum, oproj, V-transpose) rotates
through tag acc bufs=3.  Row sums accumulate on DVE in bf16 (2x mode);
output partials are written bf16 (host sums in fp32).
"""

import numpy as np
import ml_dtypes
from contextlib import ExitStack

import concourse.bass as bass
from concourse import bacc
import concourse.mybir as mybir
import concourse.tile as tile
from concourse.bass_utils import run_bass_kernel_spmd
from concourse.masks import make_identity

F32 = mybir.dt.float32
BF16 = mybir.dt.bfloat16

D = 2048
T = 2048
DH = 128
B = 2
HPC = 4            # query heads per core
NCORES = 8
NSL = 4            # t-slices of 512
SCALE = 1.0 / float(np.sqrt(128.0))

_CACHE = {}


def _build_nc():
    nc = bacc.Bacc("TRN2", target_bir_lowering=False, debug=False,
                   num_devices=NCORES)

    # packed inputs: one dram tensor per DMA group, rows are the SBUF
    # partition lines (multi-KB contiguous per row)
    xsd = [nc.dram_tensor(f"xs{j}", [128, 16 * 512], BF16,
                          kind="ExternalInput") for j in range(NSL)]
    wkd = nc.dram_tensor("wkx", [128, 16 * 128], BF16, kind="ExternalInput")
    wqd = nc.dram_tensor("wqx", [128, 16 * 512], BF16, kind="ExternalInput")
    wvd = nc.dram_tensor("wvx", [128, 16 * 128], BF16, kind="ExternalInput")
    wod = nc.dram_tensor("wox", [128, HPC * D], BF16, kind="ExternalInput")
    bqm = nc.dram_tensor("bqm", [DH, HPC], F32, kind="ExternalInput")
    bkm = nc.dram_tensor("bkm", [DH, 1], F32, kind="ExternalInput")
    bvm = nc.dram_tensor("bvm", [DH, 1], F32, kind="ExternalInput")
    part = nc.dram_tensor("part", [T, D], BF16, kind="ExternalOutput")

    with ExitStack() as ctx:
        tc = ctx.enter_context(tile.TileContext(nc))
        persist = ctx.enter_context(tc.tile_pool(name="persist", bufs=1))
        work = ctx.enter_context(tc.tile_pool(name="work", bufs=3))
        psum = ctx.enter_context(tc.tile_pool(name="psum", bufs=2, space="PSUM"))

        # ---- constants ----
        ones_bf = persist.tile([128, 128], BF16, tag="ones", name="ones_bf")
        nc.vector.memset(ones_bf, 1.0)
        ident = persist.tile([128, 128], BF16, tag="ident", name="ident")
        make_identity(nc, ident)

        bq_sb = persist.tile([DH, HPC], F32, tag="bq", name="bq_sb")
        nc.scalar.dma_start(out=bq_sb, in_=bqm[:, :])
        bk_sb = persist.tile([DH, 1], F32, tag="bk", name="bk_sb")
        nc.scalar.dma_start(out=bk_sb, in_=bkm[:, :])
        bv_sb = persist.tile([DH, 1], F32, tag="bv", name="bv_sb")
        nc.scalar.dma_start(out=bv_sb, in_=bvm[:, :])

        # ---- inputs -> SBUF on the sync HWDGE queue, in exact consumption
        # order.  Slice-0 groups are split fine-grained so the first
        # projection matmuls start as early as possible. ----
        xs0_sb = [persist.tile([128, 4 * 512], BF16, tag=f"xs0_{g}",
                               name=f"xs0_sb{g}") for g in range(4)]
        xs_sb = [None] + [persist.tile([128, 16 * 512], BF16, tag=f"xs{j}",
                                       name=f"xs_sb{j}") for j in range(1, NSL)]
        wk_sb = persist.tile([128, 16 * 128], BF16, tag="wk", name="wk_sb")
        wq2_sb = [persist.tile([128, 8 * 512], BF16, tag=f"wq{g}",
                               name=f"wq_sb{g}") for g in range(2)]
        wv_sb = persist.tile([128, 16 * 128], BF16, tag="wv", name="wv_sb")
        wo_sb = persist.tile([128, HPC * D], BF16, tag="wo", name="wo_sb")

        nc.sync.dma_start(out=wk_sb[:, 0:512], in_=wkd[:, 0:512])
        nc.sync.dma_start(out=wk_sb[:, 512:2048], in_=wkd[:, 512:2048])
        for g in range(4):
            nc.sync.dma_start(out=xs0_sb[g],
                              in_=xsd[0][:, g * 2048:(g + 1) * 2048])
        nc.sync.dma_start(out=wv_sb, in_=wvd[:, :])
        for g in range(2):
            nc.sync.dma_start(out=wq2_sb[g],
                              in_=wqd[:, g * 4096:(g + 1) * 4096])
        nc.sync.dma_start(out=xs_sb[1], in_=xsd[1][:, :])
        nc.sync.dma_start(out=wo_sb, in_=wod[:, :])
        nc.sync.dma_start(out=xs_sb[2], in_=xsd[2][:, :])
        nc.sync.dma_start(out=xs_sb[3], in_=xsd[3][:, :])

        def xrhs(j, kb):
            if j == 0:
                return xs0_sb[kb // 4][:, (kb % 4) * 512:(kb % 4 + 1) * 512]
            return xs_sb[j][:, kb * 512:(kb + 1) * 512]

        def wqap(kb, h):
            return wq2_sb[kb // 8][:, (kb % 8) * 512 + h * 128:
                                   (kb % 8) * 512 + (h + 1) * 128]

        # ---- persistent activations ----
        qT = [persist.tile([128, T], BF16, tag=f"qT{h}", name=f"qT{h}")
              for h in range(HPC)]
        kT = persist.tile([128, T], BF16, tag="kT", name="kT")
        v_sb = [persist.tile([128, DH], BF16, tag=f"v{t}", name=f"v{t}")
                for t in range(16)]
        oT = [persist.tile([128, T], BF16, tag=f"oT{h}", name=f"oT{h}")
              for h in range(HPC)]

        # ------- emission helpers; each returns a list of closures that
        # emit ONE instruction each (plus tile allocs), used as PE filler
        # interleaved into the attention stream -------

        def gen_proj(j, kind, h=0):
            """Projection group for slice j: 16 accumulating MMs + ACT
            epilogue. kind in {'k','q','v'}."""
            sl = slice(j * 512, (j + 1) * 512)
            st = {}

            def mm(kb):
                def f():
                    if kb == 0:
                        st["ps"] = psum.tile([128, 512], F32, tag="acc",
                                             bufs=3, name=f"{kind}ps{j}_{h}")
                    if kind == "k":
                        lhsT = wk_sb[:, kb * 128:(kb + 1) * 128]
                    elif kind == "v":
                        lhsT = wv_sb[:, kb * 128:(kb + 1) * 128]
                    else:
                        lhsT = wqap(kb, h)
                    nc.tensor.matmul(out=st["ps"], lhsT=lhsT,
                                     rhs=xrhs(j, kb),
                                     start=(kb == 0), stop=(kb == 15))
                return f

            steps = [mm(kb) for kb in range(16)]

            if kind == "k":
                def epi():
                    nc.scalar.activation(out=kT[:, sl], in_=st["ps"],
                                         func=mybir.ActivationFunctionType.Identity,
                                         bias=bk_sb[:, 0:1], scale=1.0)
                steps.append(epi)
            elif kind == "q":
                def epi():
                    nc.scalar.activation(out=qT[h][:, sl], in_=st["ps"],
                                         func=mybir.ActivationFunctionType.Identity,
                                         bias=bq_sb[:, h:h + 1], scale=1.0)
                steps.append(epi)
            else:
                def epi():
                    st["vt"] = work.tile([128, 512], BF16, tag="vt", bufs=2,
                                         name=f"vt{j}")
                    nc.scalar.activation(out=st["vt"], in_=st["ps"],
                                         func=mybir.ActivationFunctionType.Identity,
                                         bias=bv_sb[:, 0:1], scale=1.0)
                steps.append(epi)

                def tr(sub):
                    def f():
                        if sub == 0:
                            st["vtp"] = psum.tile([128, 512], BF16, tag="acc",
                                                  bufs=3, name=f"vtp{j}")
                        nc.tensor.transpose(
                            st["vtp"][:, sub * 128:(sub + 1) * 128],
                            st["vt"][:, sub * 128:(sub + 1) * 128], ident)
                    return f

                def cp(sub):
                    def f():
                        nc.vector.tensor_copy(
                            out=v_sb[4 * j + sub],
                            in_=st["vtp"][:, sub * 128:(sub + 1) * 128])
                    return f

                for sub in range(4):
                    steps.append(tr(sub))
                steps += [cp(sub) for sub in range(4)]
            return steps

        def gen_oproj(j, tiles=range(4)):
            """Output projection for t-tiles of slice j.  Each 512-chunk:
            4 accumulating MMs + DVE copy to bf16 staging; chunked output
            DMAs on the scalar HWDGE queue."""
            steps = []
            for tt in [4 * j + s for s in tiles]:
                st = {}

                def chunk_mm(st, tt, n, h):
                    def f():
                        if n == 0 and h == 0:
                            st["ostg"] = work.tile([128, D], BF16, tag="ostg",
                                                   bufs=2, name=f"ostg{tt}")
                        if h == 0:
                            st["ops"] = psum.tile([128, 512], F32, tag="acc",
                                                  bufs=3, name=f"ops{tt}_{n}")
                        nc.tensor.matmul(
                            out=st["ops"],
                            lhsT=oT[h][:, tt * 128:(tt + 1) * 128],
                            rhs=wo_sb[:, h * D + n * 512:h * D + (n + 1) * 512],
                            start=(h == 0), stop=(h == HPC - 1))
                    return f

                def chunk_cp(st, tt, n):
                    # alternate DVE/ACT so the oproj PSUM release never
                    # queues behind the DVE racc chains
                    def f():
                        if n % 2 == 0:
                            nc.vector.tensor_copy(
                                out=st["ostg"][:, n * 512:(n + 1) * 512],
                                in_=st["ops"])
                        else:
                            nc.scalar.copy(
                                out=st["ostg"][:, n * 512:(n + 1) * 512],
                                in_=st["ops"])
                    return f

                def out_dma(st, tt, half):
                    def f():
                        nc.scalar.dma_start(
                            out=part[tt * 128:(tt + 1) * 128,
                                     half * 1024:(half + 1) * 1024],
                            in_=st["ostg"][:, half * 1024:(half + 1) * 1024])
                    return f

                for n in range(4):
                    for h in range(HPC):
                        steps.append(chunk_mm(st, tt, n, h))
                    steps.append(chunk_cp(st, tt, n))
                    if n % 2 == 1:
                        steps.append(out_dma(st, tt, n // 2))
            return steps

        # ------- attention for slice j with filler interleave -------

        def emit_attention(j, filler):
            sl = slice(j * 512, (j + 1) * 512)
            ntk = 4 * (j + 1)
            nblocks = HPC * ntk
            bdone = 0

            def pop_filler():
                nonlocal bdone
                bdone += 1
                rem_blocks = nblocks - bdone
                if not filler:
                    return
                if rem_blocks <= 0:
                    while filler:
                        filler.pop(0)()
                    return
                k = (len(filler) + rem_blocks - 1) // rem_blocks
                for _ in range(min(k, len(filler))):
                    filler.pop(0)()

            fin_prev = [None]
            for h in range(HPC):
                st = {"sps": {}, "pt": {}}

                def score(tkb):
                    sps = psum.tile([128, 512], F32, tag="sp", bufs=3,
                                    name=f"sps{j}_{h}_{tkb}")
                    nc.tensor.matmul(out=sps,
                                     lhsT=kT[:, tkb * 128:(tkb + 1) * 128],
                                     rhs=qT[h][:, sl],
                                     start=True, stop=True)
                    pt = work.tile([128, 512], BF16, tag="pt", bufs=6,
                                   name=f"pt{j}_{h}_{tkb}")
                    nc.scalar.activation(out=pt, in_=sps,
                                         func=mybir.ActivationFunctionType.Exp,
                                         scale=SCALE)
                    if tkb >= 4 * j:
                        nc.gpsimd.affine_select(
                            out=pt, in_=pt,
                            compare_op=mybir.AluOpType.is_ge,
                            fill=0.0,
                            base=-(128 * (tkb - 4 * j)),
                            pattern=[[1, 512]],
                            channel_multiplier=-1,
                        )
                    st["pt"][tkb] = pt

                def consume(tkb):
                    pt = st["pt"].pop(tkb)
                    if tkb == 0:
                        st["otps"] = psum.tile([128, 512], F32, tag="ot",
                                               bufs=2, name=f"otps{j}_{h}")
                        st["racc"] = work.tile([128, 512], BF16, tag="racc",
                                               bufs=2, name=f"racc{j}_{h}")
                    nc.tensor.matmul(out=st["otps"], lhsT=v_sb[tkb], rhs=pt,
                                     start=(tkb == 0), stop=(tkb == ntk - 1))
                    if tkb == 0:
                        nc.vector.tensor_copy(out=st["racc"], in_=pt)
                    else:
                        nc.vector.tensor_add(out=st["racc"], in0=st["racc"],
                                             in1=pt)

                def make_fin(h, st):
                    # finalize head: rowsum via ones-matmul, normalize.
                    # Deferred into the NEXT head's stream so the rowsum MM
                    # never heads the PE queue while the DVE racc chain is
                    # still draining.
                    def fin():
                        rsb = psum.tile([128, 512], F32, tag="acc", bufs=3,
                                        name=f"rsb{j}_{h}")
                        nc.tensor.matmul(out=rsb, lhsT=ones_bf,
                                         rhs=st["racc"],
                                         start=True, stop=True)
                        rinv = work.tile([128, 512], F32, tag="rinv", bufs=2,
                                         name=f"rinv{j}_{h}")
                        nc.vector.reciprocal_approx_fast(rinv, rsb)
                        nc.vector.tensor_mul(out=oT[h][:, sl],
                                             in0=st["otps"], in1=rinv)
                    return fin

                # software pipeline: score k+3 runs ahead of AV k (covers
                # the exp + causal-mask latency of diagonal blocks)
                depth = min(3, ntk - 1)
                for tkb in range(ntk):
                    score(tkb)
                    if tkb == 1 and fin_prev[0] is not None:
                        fin_prev[0]()
                        fin_prev[0] = None
                    if tkb >= depth:
                        consume(tkb - depth)
                    pop_filler()
                for tkb in range(max(0, ntk - depth), ntk):
                    consume(tkb)
                fin_prev[0] = make_fin(h, st)

            # last head's FIN goes behind a bit of leftover filler (filler
            # never reads this slice's oT, so this is order-safe)
            for _ in range(min(6, len(filler))):
                filler.pop(0)()
            fin_prev[0]()
            fin_prev[0] = None
            # leftover filler (normally consumed inside the loop)
            while filler:
                filler.pop(0)()

        # ------- program -------
        # prologue: slice-0 projections, directly.  K and V first (their
        # weights land before wq), so the PE has work during the wq DMA.
        for f in (gen_proj(0, "k") + gen_proj(0, "v") + gen_proj(0, "q", 0)
                  + gen_proj(0, "q", 1) + gen_proj(0, "q", 2)
                  + gen_proj(0, "q", 3)):
            f()

        # filler distribution: defer half of oproj(1) to slice 3 so the
        # long slice-3 attention keeps enough PE filler
        for j in range(NSL):
            filler = []
            if j + 1 < NSL:
                filler += gen_proj(j + 1, "k")
                for h in range(HPC):
                    filler += gen_proj(j + 1, "q", h)
                filler += gen_proj(j + 1, "v")
            if j == 1:
                filler += gen_oproj(0)
            elif j == 3:
                filler += gen_oproj(1)
                filler += gen_oproj(2)
            emit_attention(j, filler)

        # epilogue: last slice's output projection
        for f in gen_oproj(NSL - 1):
            f()

    nc.compile()
    return nc


def _get_nc():
    if "nc" not in _CACHE:
        _CACHE["nc"] = _build_nc()
    return _CACHE["nc"]


def _bf16(a):
    return np.ascontiguousarray(a.astype(ml_dtypes.bfloat16))


def kernel(x, Wq, bq, Wk, bk, Wv, bv, Wo, bo, **kw):
    x = np.asarray(x, dtype=np.float32)
    Wq = np.asarray(Wq, dtype=np.float32)
    Wk = np.asarray(Wk, dtype=np.float32)
    Wv = np.asarray(Wv, dtype=np.float32)
    Wo = np.asarray(Wo, dtype=np.float32)
    bq = np.asarray(bq, dtype=np.float32)
    bk = np.asarray(bk, dtype=np.float32)
    bv = np.asarray(bv, dtype=np.float32)
    bo = np.asarray(bo, dtype=np.float32)

    nc = _get_nc()

    # x slices, shared per batch: xs[b][j] = [128, 16*512] with columns
    # (kb, t') st. xs[b][j][p, kb*512+t'] = x[b, j*512+t', kb*128+p]
    xs_b = []
    for b in range(B):
        xT = np.ascontiguousarray(x[b].T)            # [D, T]
        xs = xT.reshape(16, 128, NSL, 512).transpose(2, 1, 0, 3)
        xs_b.append(_bf16(xs.reshape(NSL, 128, 16 * 512)))

    # per head-quarter weight packs, shared across batches
    packs = []
    for q in range(HPC):
        hs = q * HPC * DH
        kv = q // 2
        wqp = _bf16(Wq[:, hs:hs + HPC * DH].reshape(16, 128, HPC * DH)
                    .transpose(1, 0, 2).reshape(128, 16 * 512))
        wkp = _bf16(Wk[:, kv * DH:(kv + 1) * DH].reshape(16, 128, DH)
                    .transpose(1, 0, 2).reshape(128, 16 * 128))
        wvp = _bf16(Wv[:, kv * DH:(kv + 1) * DH].reshape(16, 128, DH)
                    .transpose(1, 0, 2).reshape(128, 16 * 128))
        wop = _bf16(Wo[hs:hs + HPC * DH, :].reshape(HPC, 128, D)
                    .transpose(1, 0, 2).reshape(128, HPC * D))
        bq_m = np.ascontiguousarray(
            bq[hs:hs + HPC * DH].reshape(HPC, DH).T)          # [128, 4]
        bk_m = np.ascontiguousarray(
            bk[kv * DH:(kv + 1) * DH].reshape(DH, 1))         # [128, 1]
        bv_m = np.ascontiguousarray(
            bv[kv * DH:(kv + 1) * DH].reshape(DH, 1))         # [128, 1]
        packs.append((wqp, wkp, wvp, wop, bq_m, bk_m, bv_m))

    in_maps = []
    for c in range(NCORES):
        b = c // 4
        q = c % 4
        wqp, wkp, wvp, wop, bq_m, bk_m, bv_m = packs[q]
        m = {f"xs{j}": xs_b[b][j] for j in range(NSL)}
        m.update({
            "wqx": wqp, "wkx": wkp, "wvx": wvp, "wox": wop,
            "bqm": bq_m, "bkm": bk_m, "bvm": bv_m,
        })
        in_maps.append(m)

    res = run_bass_kernel_spmd(nc, in_maps, list(range(NCORES)),
                               **kw.get("_run_kwargs", {}))
    if kw.get("_return_res"):
        return res
    parts = [res.results[c]["part"] for c in range(NCORES)]
    out = np.empty((B, T, D), dtype=np.float32)
    for b in range(B):
        acc = parts[4 * b].astype(np.float32)
        for q in range(1, 4):
            acc = acc + parts[4 * b + q].astype(np.float32)
        out[b] = acc + bo[None, :]
    return out


# revision 21
# speedup vs baseline: 1.5153x; 1.0473x over previous
"""GQA kernel for Trainium2, 8 NeuronCores.

Problem: B=2, T=2048, D=2048, 16 query heads / 2 KV heads, d_head=128, causal.

Sharding: core c -> batch b = c//4, head-quarter q = c%4 (query heads
4q..4q+3, kv head q//2). Each core computes its 4 heads' attention and a
partial output projection (its Wo rows); host sums the 4 partials per batch
and adds bo.

Host marshalling: all inputs pre-cast to bf16 and packed into [128, N]
arrays whose column layout equals the SBUF tile layout, so each logical
group is one large DMA with multi-KB contiguous rows, issued on a single
HWDGE queue in exact consumption order:
  wk, x-slice0 (4 chunks), wv, wq (2 chunks), x-slice1, wo, x-s2, x-s3.

On-core dataflow (bf16 matmuls, fp32 PSUM accum), 4 rounds over 512-wide
t-slices. Per slice j the ACT-heavy attention blocks (score -> exp ->
[causal mask] -> AV) are emitted software-pipelined (score k+3 ahead of
AV k, covering exp + mask latency) with the PE-dense filler work
(projections of slice j+1, output projection of an earlier slice) spread
between them a few matmuls at a time, so the statically-ordered PE queue
never waits on ACT/DVE progress.  Each head's finalize (rowsum matmul,
reciprocal, normalize) is deferred into the next head's stream so it
never heads the PE queue while the DVE row-sum chain drains.

PSUM: sps bufs=3 (tag sp), otps bufs=2 (tag ot, long-lived across the tk
loop), everything else (proj accums, rowsum, oproj, V-transpose) rotates
through tag acc bufs=3.  Row sums accumulate on DVE in bf16 (2x mode);
output partials are written bf16 (host sums in fp32).
"""

import numpy as np
import ml_dtypes
from contextlib import ExitStack

import concourse.bass as bass
from concourse import bacc
import concourse.mybir as mybir
import concourse.tile as tile
from concourse.bass_utils import run_bass_kernel_spmd
from concourse.masks import make_identity

F32 = mybir.dt.float32
BF16 = mybir.dt.bfloat16

D = 2048
T = 2048
DH = 128
B = 2
HPC = 4            # query heads per core
NCORES = 8
NSL = 4            # t-slices of 512
SCALE = 1.0 / float(np.sqrt(128.0))

_CACHE = {}


def _build_nc():
    nc = bacc.Bacc("TRN2", target_bir_lowering=False, debug=False,
                   num_devices=NCORES)

    # packed inputs: one dram tensor per DMA group, rows are the SBUF
    # partition lines (multi-KB contiguous per row)
    xsd = [nc.dram_tensor(f"xs{j}", [128, 16 * 512], BF16,
                          kind="ExternalInput") for j in range(NSL)]
    wkd = nc.dram_tensor("wkx", [128, 16 * 128], BF16, kind="ExternalInput")
    wqd = nc.dram_tensor("wqx", [128, 16 * 512], BF16, kind="ExternalInput")
    wvd = nc.dram_tensor("wvx", [128, 16 * 128], BF16, kind="ExternalInput")
    wod = nc.dram_tensor("wox", [128, HPC * D], BF16, kind="ExternalInput")
    bqm = nc.dram_tensor("bqm", [DH, HPC], F32, kind="ExternalInput")
    bkm = nc.dram_tensor("bkm", [DH, 1], F32, kind="ExternalInput")
    bvm = nc.dram_tensor("bvm", [DH, 1], F32, kind="ExternalInput")
    part = nc.dram_tensor("part", [T, D], BF16, kind="ExternalOutput")

    with ExitStack() as ctx:
        tc = ctx.enter_context(tile.TileContext(nc))
        persist = ctx.enter_context(tc.tile_pool(name="persist", bufs=1))
        work = ctx.enter_context(tc.tile_pool(name="work", bufs=3))
        psum = ctx.enter_context(tc.tile_pool(name="psum", bufs=2, space="PSUM"))

        # ---- constants ----
        ones_bf = persist.tile([128, 128], BF16, tag="ones", name="ones_bf")
        nc.vector.memset(ones_bf, 1.0)
        ident = persist.tile([128, 128], BF16, tag="ident", name="ident")
        make_identity(nc, ident)

        bq_sb = persist.tile([DH, HPC], F32, tag="bq", name="bq_sb")
        nc.scalar.dma_start(out=bq_sb, in_=bqm[:, :])
        bk_sb = persist.tile([DH, 1], F32, tag="bk", name="bk_sb")
        nc.scalar.dma_start(out=bk_sb, in_=bkm[:, :])
        bv_sb = persist.tile([DH, 1], F32, tag="bv", name="bv_sb")
        nc.scalar.dma_start(out=bv_sb, in_=bvm[:, :])

        # ---- inputs -> SBUF on the sync HWDGE queue, in exact consumption
        # order.  Slice-0 groups are split fine-grained so the first
        # projection matmuls start as early as possible. ----
        xs0_sb = [persist.tile([128, 4 * 512], BF16, tag=f"xs0_{g}",
                               name=f"xs0_sb{g}") for g in range(4)]
        xs_sb = [None] + [persist.tile([128, 16 * 512], BF16, tag=f"xs{j}",
                                       name=f"xs_sb{j}") for j in range(1, NSL)]
        wk_sb = persist.tile([128, 16 * 128], BF16, tag="wk", name="wk_sb")
        wq2_sb = [persist.tile([128, 8 * 512], BF16, tag=f"wq{g}",
                               name=f"wq_sb{g}") for g in range(2)]
        wv_sb = persist.tile([128, 16 * 128], BF16, tag="wv", name="wv_sb")
        wo_sb = persist.tile([128, HPC * D], BF16, tag="wo", name="wo_sb")

        nc.sync.dma_start(out=wk_sb[:, 0:512], in_=wkd[:, 0:512])
        nc.sync.dma_start(out=wk_sb[:, 512:2048], in_=wkd[:, 512:2048])
        for g in range(2):
            nc.sync.dma_start(out=xs0_sb[g],
                              in_=xsd[0][:, g * 2048:(g + 1) * 2048])
        nc.sync.dma_start(out=wv_sb, in_=wvd[:, :])
        for g in range(2, 4):
            nc.sync.dma_start(out=xs0_sb[g],
                              in_=xsd[0][:, g * 2048:(g + 1) * 2048])
        for g in range(2):
            nc.sync.dma_start(out=wq2_sb[g],
                              in_=wqd[:, g * 4096:(g + 1) * 4096])
        nc.sync.dma_start(out=xs_sb[1], in_=xsd[1][:, :])
        nc.sync.dma_start(out=wo_sb, in_=wod[:, :])
        nc.sync.dma_start(out=xs_sb[2], in_=xsd[2][:, :])
        nc.sync.dma_start(out=xs_sb[3], in_=xsd[3][:, :])

        def xrhs(j, kb):
            if j == 0:
                return xs0_sb[kb // 4][:, (kb % 4) * 512:(kb % 4 + 1) * 512]
            return xs_sb[j][:, kb * 512:(kb + 1) * 512]

        def wqap(kb, h):
            return wq2_sb[kb // 8][:, (kb % 8) * 512 + h * 128:
                                   (kb % 8) * 512 + (h + 1) * 128]

        # ---- persistent activations ----
        qT = [persist.tile([128, T], BF16, tag=f"qT{h}", name=f"qT{h}")
              for h in range(HPC)]
        kT = persist.tile([128, T], BF16, tag="kT", name="kT")
        v_sb = [persist.tile([128, DH], BF16, tag=f"v{t}", name=f"v{t}")
                for t in range(16)]
        oT = [persist.tile([128, T], BF16, tag=f"oT{h}", name=f"oT{h}")
              for h in range(HPC)]

        # ------- emission helpers; each returns a list of closures that
        # emit ONE instruction each (plus tile allocs), used as PE filler
        # interleaved into the attention stream -------

        def gen_proj(j, kind, h=0):
            """Projection group for slice j: 16 accumulating MMs + ACT
            epilogue. kind in {'k','q','v'}."""
            sl = slice(j * 512, (j + 1) * 512)
            st = {}

            def mm(kb):
                def f():
                    if kb == 0:
                        st["ps"] = psum.tile([128, 512], F32, tag="acc",
                                             bufs=3, name=f"{kind}ps{j}_{h}")
                    if kind == "k":
                        lhsT = wk_sb[:, kb * 128:(kb + 1) * 128]
                    elif kind == "v":
                        lhsT = wv_sb[:, kb * 128:(kb + 1) * 128]
                    else:
                        lhsT = wqap(kb, h)
                    nc.tensor.matmul(out=st["ps"], lhsT=lhsT,
                                     rhs=xrhs(j, kb),
                                     start=(kb == 0), stop=(kb == 15))
                return f

            steps = [mm(kb) for kb in range(16)]

            if kind == "k":
                def epi():
                    nc.scalar.activation(out=kT[:, sl], in_=st["ps"],
                                         func=mybir.ActivationFunctionType.Identity,
                                         bias=bk_sb[:, 0:1], scale=1.0)
                steps.append(epi)
            elif kind == "q":
                def epi():
                    nc.scalar.activation(out=qT[h][:, sl], in_=st["ps"],
                                         func=mybir.ActivationFunctionType.Identity,
                                         bias=bq_sb[:, h:h + 1], scale=1.0)
                steps.append(epi)
            else:
                def epi():
                    st["vt"] = work.tile([128, 512], BF16, tag="vt", bufs=2,
                                         name=f"vt{j}")
                    nc.scalar.activation(out=st["vt"], in_=st["ps"],
                                         func=mybir.ActivationFunctionType.Identity,
                                         bias=bv_sb[:, 0:1], scale=1.0)
                steps.append(epi)

                def tr(sub):
                    def f():
                        if sub == 0:
                            st["vtp"] = psum.tile([128, 512], BF16, tag="acc",
                                                  bufs=3, name=f"vtp{j}")
                        nc.tensor.transpose(
                            st["vtp"][:, sub * 128:(sub + 1) * 128],
                            st["vt"][:, sub * 128:(sub + 1) * 128], ident)
                    return f

                def cp(sub):
                    def f():
                        nc.vector.tensor_copy(
                            out=v_sb[4 * j + sub],
                            in_=st["vtp"][:, sub * 128:(sub + 1) * 128])
                    return f

                for sub in range(4):
                    steps.append(tr(sub))
                steps += [cp(sub) for sub in range(4)]
            return steps

        def gen_oproj(j, tiles=range(4)):
            """Output projection for t-tiles of slice j.  Each 512-chunk:
            4 accumulating MMs + DVE copy to bf16 staging; chunked output
            DMAs on the scalar HWDGE queue."""
            steps = []
            for tt in [4 * j + s for s in tiles]:
                st = {}

                def chunk_mm(st, tt, n, h):
                    def f():
                        if n == 0 and h == 0:
                            st["ostg"] = work.tile([128, D], BF16, tag="ostg",
                                                   bufs=2, name=f"ostg{tt}")
                        if h == 0:
                            st["ops"] = psum.tile([128, 512], F32, tag="acc",
                                                  bufs=3, name=f"ops{tt}_{n}")
                        nc.tensor.matmul(
                            out=st["ops"],
                            lhsT=oT[h][:, tt * 128:(tt + 1) * 128],
                            rhs=wo_sb[:, h * D + n * 512:h * D + (n + 1) * 512],
                            start=(h == 0), stop=(h == HPC - 1))
                    return f

                def chunk_cp(st, tt, n):
                    # alternate DVE/ACT so the oproj PSUM release never
                    # queues behind the DVE racc chains
                    def f():
                        if n % 2 == 0:
                            nc.vector.tensor_copy(
                                out=st["ostg"][:, n * 512:(n + 1) * 512],
                                in_=st["ops"])
                        else:
                            nc.scalar.copy(
                                out=st["ostg"][:, n * 512:(n + 1) * 512],
                                in_=st["ops"])
                    return f

                def out_dma(st, tt, half):
                    def f():
                        nc.scalar.dma_start(
                            out=part[tt * 128:(tt + 1) * 128,
                                     half * 1024:(half + 1) * 1024],
                            in_=st["ostg"][:, half * 1024:(half + 1) * 1024])
                    return f

                for n in range(4):
                    for h in range(HPC):
                        steps.append(chunk_mm(st, tt, n, h))
                    steps.append(chunk_cp(st, tt, n))
                    if n % 2 == 1:
                        steps.append(out_dma(st, tt, n // 2))
            return steps

        # ------- attention for slice j with filler interleave -------

        def emit_attention(j, filler):
            sl = slice(j * 512, (j + 1) * 512)
            ntk = 4 * (j + 1)
            nblocks = HPC * ntk
            bdone = 0

            def pop_filler():
                nonlocal bdone
                bdone += 1
                rem_blocks = nblocks - bdone
                if not filler:
                    return
                if rem_blocks <= 0:
                    while filler:
                        filler.pop(0)()
                    return
                k = (len(filler) + rem_blocks - 1) // rem_blocks
                for _ in range(min(k, len(filler))):
                    filler.pop(0)()

            fin_prev = [None]
            for h in range(HPC):
                st = {"sps": {}, "pt": {}}

                def score(tkb):
                    # causal trim: diagonal tk-block 4j+s only needs
                    # tq_local >= 128*s, so score/exp/AV run on [lo:512]
                    s = tkb - 4 * j
                    lo = 128 * s if s >= 0 else 0
                    sps = psum.tile([128, 512], F32, tag="sp", bufs=3,
                                    name=f"sps{j}_{h}_{tkb}")
                    nc.tensor.matmul(out=sps[:, lo:512],
                                     lhsT=kT[:, tkb * 128:(tkb + 1) * 128],
                                     rhs=qT[h][:, j * 512 + lo:(j + 1) * 512],
                                     start=True, stop=True)
                    pt = work.tile([128, 512], BF16, tag="pt", bufs=6,
                                   name=f"pt{j}_{h}_{tkb}")
                    nc.scalar.activation(out=pt[:, lo:512],
                                         in_=sps[:, lo:512],
                                         func=mybir.ActivationFunctionType.Exp,
                                         scale=SCALE)
                    if s >= 0:
                        # strict upper triangle of the 128x128 tile at the
                        # diagonal: zero where local tq < tk
                        nc.gpsimd.affine_select(
                            out=pt[:, lo:lo + 128], in_=pt[:, lo:lo + 128],
                            compare_op=mybir.AluOpType.is_ge,
                            fill=0.0,
                            base=0,
                            pattern=[[1, 128]],
                            channel_multiplier=-1,
                        )
                    st["pt"][tkb] = (pt, lo)

                def consume(tkb):
                    pt, lo = st["pt"].pop(tkb)
                    if tkb == 0:
                        st["otps"] = psum.tile([128, 512], F32, tag="ot",
                                               bufs=2, name=f"otps{j}_{h}")
                        st["racc"] = work.tile([128, 512], BF16, tag="racc",
                                               bufs=2, name=f"racc{j}_{h}")
                    nc.tensor.matmul(out=st["otps"][:, lo:512],
                                     lhsT=v_sb[tkb], rhs=pt[:, lo:512],
                                     start=(tkb == 0), stop=(tkb == ntk - 1))
                    if tkb == 0:
                        nc.vector.tensor_copy(out=st["racc"], in_=pt)
                    else:
                        nc.vector.tensor_add(out=st["racc"][:, lo:512],
                                             in0=st["racc"][:, lo:512],
                                             in1=pt[:, lo:512])

                def make_fin(h, st):
                    # finalize head: rowsum via ones-matmul, normalize.
                    # Deferred into the NEXT head's stream so the rowsum MM
                    # never heads the PE queue while the DVE racc chain is
                    # still draining.
                    def fin():
                        rsb = psum.tile([128, 512], F32, tag="acc", bufs=3,
                                        name=f"rsb{j}_{h}")
                        nc.tensor.matmul(out=rsb, lhsT=ones_bf,
                                         rhs=st["racc"],
                                         start=True, stop=True)
                        rinv = work.tile([128, 512], F32, tag="rinv", bufs=2,
                                         name=f"rinv{j}_{h}")
                        nc.vector.reciprocal_approx_fast(rinv, rsb)
                        nc.vector.tensor_mul(out=oT[h][:, sl],
                                             in0=st["otps"], in1=rinv)
                    return fin

                # software pipeline: score k+3 runs ahead of AV k (covers
                # the exp + causal-mask latency of diagonal blocks)
                depth = min(3, ntk - 1)
                for tkb in range(ntk):
                    score(tkb)
                    if tkb == 1 and fin_prev[0] is not None:
                        fin_prev[0]()
                        fin_prev[0] = None
                    if tkb >= depth:
                        consume(tkb - depth)
                    pop_filler()
                for tkb in range(max(0, ntk - depth), ntk):
                    consume(tkb)
                fin_prev[0] = make_fin(h, st)

            # last head's FIN goes behind a bit of leftover filler (filler
            # never reads this slice's oT, so this is order-safe)
            for _ in range(min(6, len(filler))):
                filler.pop(0)()
            fin_prev[0]()
            fin_prev[0] = None
            # leftover filler (normally consumed inside the loop)
            while filler:
                filler.pop(0)()

        # ------- program -------
        # prologue: slice-0 projections, ordered to track the DMA stream.
        # K and V interleave per xs0 chunk (their weights land before wq)
        # so the PE has maximal ready work per landed byte; Q heads 0/1 run
        # their wq_a halves first so they never wait on wq_b.
        K0 = gen_proj(0, "k")
        V0 = gen_proj(0, "v")
        Qs = [gen_proj(0, "q", hh) for hh in range(HPC)]
        for f in (K0[0:8] + V0[0:8] + K0[8:12] + V0[8:12]
                  + K0[12:16] + V0[12:16] + [K0[16]] + V0[16:]
                  + Qs[0][0:8] + Qs[1][0:8] + Qs[0][8:17] + Qs[1][8:17]
                  + Qs[2] + Qs[3]):
            f()

        # filler distribution: defer half of oproj(1) to slice 3 so the
        # long slice-3 attention keeps enough PE filler
        for j in range(NSL):
            filler = []
            if j + 1 < NSL:
                filler += gen_proj(j + 1, "k")
                for h in range(HPC):
                    filler += gen_proj(j + 1, "q", h)
                filler += gen_proj(j + 1, "v")
            if j == 1:
                filler += gen_oproj(0)
            elif j == 3:
                filler += gen_oproj(1)
                filler += gen_oproj(2)
            emit_attention(j, filler)

        # epilogue: last slice's output projection
        for f in gen_oproj(NSL - 1):
            f()

    nc.compile()
    return nc


def _get_nc():
    if "nc" not in _CACHE:
        _CACHE["nc"] = _build_nc()
    return _CACHE["nc"]


def _bf16(a):
    return np.ascontiguousarray(a.astype(ml_dtypes.bfloat16))


def kernel(x, Wq, bq, Wk, bk, Wv, bv, Wo, bo, **kw):
    x = np.asarray(x, dtype=np.float32)
    Wq = np.asarray(Wq, dtype=np.float32)
    Wk = np.asarray(Wk, dtype=np.float32)
    Wv = np.asarray(Wv, dtype=np.float32)
    Wo = np.asarray(Wo, dtype=np.float32)
    bq = np.asarray(bq, dtype=np.float32)
    bk = np.asarray(bk, dtype=np.float32)
    bv = np.asarray(bv, dtype=np.float32)
    bo = np.asarray(bo, dtype=np.float32)

    nc = _get_nc()

    # x slices, shared per batch: xs[b][j] = [128, 16*512] with columns
    # (kb, t') st. xs[b][j][p, kb*512+t'] = x[b, j*512+t', kb*128+p]
    xs_b = []
    for b in range(B):
        xT = np.ascontiguousarray(x[b].T)            # [D, T]
        xs = xT.reshape(16, 128, NSL, 512).transpose(2, 1, 0, 3)
        xs_b.append(_bf16(xs.reshape(NSL, 128, 16 * 512)))

    # per head-quarter weight packs, shared across batches
    packs = []
    for q in range(HPC):
        hs = q * HPC * DH
        kv = q // 2
        wqp = _bf16(Wq[:, hs:hs + HPC * DH].reshape(16, 128, HPC * DH)
                    .transpose(1, 0, 2).reshape(128, 16 * 512))
        wkp = _bf16(Wk[:, kv * DH:(kv + 1) * DH].reshape(16, 128, DH)
                    .transpose(1, 0, 2).reshape(128, 16 * 128))
        wvp = _bf16(Wv[:, kv * DH:(kv + 1) * DH].reshape(16, 128, DH)
                    .transpose(1, 0, 2).reshape(128, 16 * 128))
        wop = _bf16(Wo[hs:hs + HPC * DH, :].reshape(HPC, 128, D)
                    .transpose(1, 0, 2).reshape(128, HPC * D))
        bq_m = np.ascontiguousarray(
            bq[hs:hs + HPC * DH].reshape(HPC, DH).T)          # [128, 4]
        bk_m = np.ascontiguousarray(
            bk[kv * DH:(kv + 1) * DH].reshape(DH, 1))         # [128, 1]
        bv_m = np.ascontiguousarray(
            bv[kv * DH:(kv + 1) * DH].reshape(DH, 1))         # [128, 1]
        packs.append((wqp, wkp, wvp, wop, bq_m, bk_m, bv_m))

    in_maps = []
    for c in range(NCORES):
        b = c // 4
        q = c % 4
        wqp, wkp, wvp, wop, bq_m, bk_m, bv_m = packs[q]
        m = {f"xs{j}": xs_b[b][j] for j in range(NSL)}
        m.update({
            "wqx": wqp, "wkx": wkp, "wvx": wvp, "wox": wop,
            "bqm": bq_m, "bkm": bk_m, "bvm": bv_m,
        })
        in_maps.append(m)

    res = run_bass_kernel_spmd(nc, in_maps, list(range(NCORES)),
                               **kw.get("_run_kwargs", {}))
    if kw.get("_return_res"):
        return res
    parts = [res.results[c]["part"] for c in range(NCORES)]
    out = np.empty((B, T, D), dtype=np.float32)
    for b in range(B):
        acc = parts[4 * b].astype(np.float32)
        for q in range(1, 4):
            acc = acc + parts[4 * b + q].astype(np.float32)
        out[b] = acc + bo[None, :]
    return out


# revision 24
# speedup vs baseline: 1.5211x; 1.0038x over previous
"""GQA kernel for Trainium2, 8 NeuronCores.

Problem: B=2, T=2048, D=2048, 16 query heads / 2 KV heads, d_head=128, causal.

Sharding: core c -> batch b = c//4, head-quarter q = c%4 (query heads
4q..4q+3, kv head q//2). Each core computes its 4 heads' attention and a
partial output projection (its Wo rows); host sums the 4 partials per batch
and adds bo.

Host marshalling: all inputs pre-cast to bf16 and packed into [128, N]
arrays whose column layout equals the SBUF tile layout, so each logical
group is one large DMA with multi-KB contiguous rows, issued on a single
HWDGE queue in exact consumption order:
  wk, x-slice0 (4 chunks), wv, wq (2 chunks), x-slice1, wo, x-s2, x-s3.

On-core dataflow (bf16 matmuls, fp32 PSUM accum), 4 rounds over 512-wide
t-slices. Per slice j the ACT-heavy attention blocks (score -> exp ->
[causal mask] -> AV) are emitted software-pipelined (score k+3 ahead of
AV k, covering exp + mask latency) with the PE-dense filler work
(projections of slice j+1, output projection of an earlier slice) spread
between them a few matmuls at a time, so the statically-ordered PE queue
never waits on ACT/DVE progress.  Each head's finalize (rowsum matmul,
reciprocal, normalize) is deferred into the next head's stream so it
never heads the PE queue while the DVE row-sum chain drains.

PSUM: sps bufs=3 (tag sp), otps bufs=2 (tag ot, long-lived across the tk
loop), everything else (proj accums, rowsum, oproj, V-transpose) rotates
through tag acc bufs=3.  Row sums accumulate on DVE in bf16 (2x mode);
output partials are written bf16 (host sums in fp32).
"""

import numpy as np
import ml_dtypes
from contextlib import ExitStack

import concourse.bass as bass
from concourse import bacc
import concourse.mybir as mybir
import concourse.tile as tile
from concourse.bass_utils import run_bass_kernel_spmd
from concourse.masks import make_identity

F32 = mybir.dt.float32
BF16 = mybir.dt.bfloat16

D = 2048
T = 2048
DH = 128
B = 2
HPC = 4            # query heads per core
NCORES = 8
NSL = 4            # t-slices of 512
SCALE = 1.0 / float(np.sqrt(128.0))

_CACHE = {}


def _build_nc():
    nc = bacc.Bacc("TRN2", target_bir_lowering=False, debug=False,
                   num_devices=NCORES)

    # packed inputs: one dram tensor per DMA group, rows are the SBUF
    # partition lines (multi-KB contiguous per row)
    xsd = [nc.dram_tensor(f"xs{j}", [128, 16 * 512], BF16,
                          kind="ExternalInput") for j in range(NSL)]
    wkd = nc.dram_tensor("wkx", [128, 16 * 128], BF16, kind="ExternalInput")
    wqd = nc.dram_tensor("wqx", [128, 16 * 512], BF16, kind="ExternalInput")
    wvd = nc.dram_tensor("wvx", [128, 16 * 128], BF16, kind="ExternalInput")
    wod = nc.dram_tensor("wox", [128, HPC * D], BF16, kind="ExternalInput")
    bqm = nc.dram_tensor("bqm", [DH, HPC], F32, kind="ExternalInput")
    bkm = nc.dram_tensor("bkm", [DH, 1], F32, kind="ExternalInput")
    bvm = nc.dram_tensor("bvm", [DH, 1], F32, kind="ExternalInput")
    part = nc.dram_tensor("part", [T, D], BF16, kind="ExternalOutput")

    with ExitStack() as ctx:
        tc = ctx.enter_context(tile.TileContext(nc))
        persist = ctx.enter_context(tc.tile_pool(name="persist", bufs=1))
        work = ctx.enter_context(tc.tile_pool(name="work", bufs=3))
        psum = ctx.enter_context(tc.tile_pool(name="psum", bufs=2, space="PSUM"))

        # ---- constants ----
        ones_bf = persist.tile([128, 128], BF16, tag="ones", name="ones_bf")
        nc.vector.memset(ones_bf, 1.0)
        ident = persist.tile([128, 128], BF16, tag="ident", name="ident")
        make_identity(nc, ident)

        bq_sb = persist.tile([DH, HPC], F32, tag="bq", name="bq_sb")
        nc.scalar.dma_start(out=bq_sb, in_=bqm[:, :])
        bk_sb = persist.tile([DH, 1], F32, tag="bk", name="bk_sb")
        nc.scalar.dma_start(out=bk_sb, in_=bkm[:, :])
        bv_sb = persist.tile([DH, 1], F32, tag="bv", name="bv_sb")
        nc.scalar.dma_start(out=bv_sb, in_=bvm[:, :])

        # ---- inputs -> SBUF on the sync HWDGE queue, in exact consumption
        # order.  Slice-0 groups are split fine-grained so the first
        # projection matmuls start as early as possible. ----
        xs0_sb = [persist.tile([128, 2 * 512], BF16, tag=f"xs0_{g}",
                               name=f"xs0_sb{g}") for g in range(8)]
        xs_sb = [None] + [persist.tile([128, 16 * 512], BF16, tag=f"xs{j}",
                                       name=f"xs_sb{j}") for j in range(1, NSL)]
        wk_sb = persist.tile([128, 16 * 128], BF16, tag="wk", name="wk_sb")
        wq2_sb = [persist.tile([128, 8 * 512], BF16, tag=f"wq{g}",
                               name=f"wq_sb{g}") for g in range(2)]
        wv_sb = persist.tile([128, 16 * 128], BF16, tag="wv", name="wv_sb")
        wo_sb = persist.tile([128, HPC * D], BF16, tag="wo", name="wo_sb")

        nc.sync.dma_start(out=wk_sb[:, 0:512], in_=wkd[:, 0:512])
        nc.sync.dma_start(out=wk_sb[:, 512:2048], in_=wkd[:, 512:2048])
        for g in range(4):
            nc.sync.dma_start(out=xs0_sb[g],
                              in_=xsd[0][:, g * 1024:(g + 1) * 1024])
        nc.sync.dma_start(out=wv_sb, in_=wvd[:, :])
        for g in range(4, 8):
            nc.sync.dma_start(out=xs0_sb[g],
                              in_=xsd[0][:, g * 1024:(g + 1) * 1024])
        for g in range(2):
            nc.sync.dma_start(out=wq2_sb[g],
                              in_=wqd[:, g * 4096:(g + 1) * 4096])
        nc.sync.dma_start(out=xs_sb[1], in_=xsd[1][:, :])
        nc.sync.dma_start(out=wo_sb, in_=wod[:, :])
        nc.sync.dma_start(out=xs_sb[2], in_=xsd[2][:, :])
        nc.sync.dma_start(out=xs_sb[3], in_=xsd[3][:, :])

        def xrhs(j, kb):
            if j == 0:
                return xs0_sb[kb // 2][:, (kb % 2) * 512:(kb % 2 + 1) * 512]
            return xs_sb[j][:, kb * 512:(kb + 1) * 512]

        def wqap(kb, h):
            return wq2_sb[kb // 8][:, (kb % 8) * 512 + h * 128:
                                   (kb % 8) * 512 + (h + 1) * 128]

        # ---- persistent activations ----
        qT = [persist.tile([128, T], BF16, tag=f"qT{h}", name=f"qT{h}")
              for h in range(HPC)]
        kT = persist.tile([128, T], BF16, tag="kT", name="kT")
        v_sb = [persist.tile([128, DH], BF16, tag=f"v{t}", name=f"v{t}")
                for t in range(16)]
        oT = [persist.tile([128, T], BF16, tag=f"oT{h}", name=f"oT{h}")
              for h in range(HPC)]

        # ------- emission helpers; each returns a list of closures that
        # emit ONE instruction each (plus tile allocs), used as PE filler
        # interleaved into the attention stream -------

        def gen_proj(j, kind, h=0):
            """Projection group for slice j: 16 accumulating MMs + ACT
            epilogue. kind in {'k','q','v'}."""
            sl = slice(j * 512, (j + 1) * 512)
            st = {}

            def mm(kb):
                def f():
                    if kb == 0:
                        st["ps"] = psum.tile([128, 512], F32, tag="acc",
                                             bufs=3, name=f"{kind}ps{j}_{h}")
                    if kind == "k":
                        lhsT = wk_sb[:, kb * 128:(kb + 1) * 128]
                    elif kind == "v":
                        lhsT = wv_sb[:, kb * 128:(kb + 1) * 128]
                    else:
                        lhsT = wqap(kb, h)
                    nc.tensor.matmul(out=st["ps"], lhsT=lhsT,
                                     rhs=xrhs(j, kb),
                                     start=(kb == 0), stop=(kb == 15))
                return f

            steps = [mm(kb) for kb in range(16)]

            if kind == "k":
                def epi():
                    nc.scalar.activation(out=kT[:, sl], in_=st["ps"],
                                         func=mybir.ActivationFunctionType.Identity,
                                         bias=bk_sb[:, 0:1], scale=1.0)
                steps.append(epi)
            elif kind == "q":
                def epi():
                    nc.scalar.activation(out=qT[h][:, sl], in_=st["ps"],
                                         func=mybir.ActivationFunctionType.Identity,
                                         bias=bq_sb[:, h:h + 1], scale=1.0)
                steps.append(epi)
            else:
                def epi():
                    st["vt"] = work.tile([128, 512], BF16, tag="vt", bufs=2,
                                         name=f"vt{j}")
                    nc.scalar.activation(out=st["vt"], in_=st["ps"],
                                         func=mybir.ActivationFunctionType.Identity,
                                         bias=bv_sb[:, 0:1], scale=1.0)
                steps.append(epi)

                def tr(sub):
                    def f():
                        if sub == 0:
                            st["vtp"] = psum.tile([128, 512], BF16, tag="acc",
                                                  bufs=3, name=f"vtp{j}")
                        nc.tensor.transpose(
                            st["vtp"][:, sub * 128:(sub + 1) * 128],
                            st["vt"][:, sub * 128:(sub + 1) * 128], ident)
                    return f

                def cp(sub):
                    def f():
                        nc.vector.tensor_copy(
                            out=v_sb[4 * j + sub],
                            in_=st["vtp"][:, sub * 128:(sub + 1) * 128])
                    return f

                for sub in range(4):
                    steps.append(tr(sub))
                steps += [cp(sub) for sub in range(4)]
            return steps

        def gen_oproj(j, tiles=range(4)):
            """Output projection for t-tiles of slice j.  Each 512-chunk:
            4 accumulating MMs + DVE copy to bf16 staging; chunked output
            DMAs on the scalar HWDGE queue."""
            steps = []
            for tt in [4 * j + s for s in tiles]:
                st = {}

                def chunk_mm(st, tt, n, h):
                    def f():
                        if n == 0 and h == 0:
                            st["ostg"] = work.tile([128, D], BF16, tag="ostg",
                                                   bufs=2, name=f"ostg{tt}")
                        if h == 0:
                            st["ops"] = psum.tile([128, 512], F32, tag="acc",
                                                  bufs=3, name=f"ops{tt}_{n}")
                        nc.tensor.matmul(
                            out=st["ops"],
                            lhsT=oT[h][:, tt * 128:(tt + 1) * 128],
                            rhs=wo_sb[:, h * D + n * 512:h * D + (n + 1) * 512],
                            start=(h == 0), stop=(h == HPC - 1))
                    return f

                def chunk_cp(st, tt, n):
                    # alternate DVE/ACT so the oproj PSUM release never
                    # queues behind the DVE racc chains
                    def f():
                        if n % 2 == 0:
                            nc.vector.tensor_copy(
                                out=st["ostg"][:, n * 512:(n + 1) * 512],
                                in_=st["ops"])
                        else:
                            nc.scalar.copy(
                                out=st["ostg"][:, n * 512:(n + 1) * 512],
                                in_=st["ops"])
                    return f

                def out_dma(st, tt, half):
                    def f():
                        nc.scalar.dma_start(
                            out=part[tt * 128:(tt + 1) * 128,
                                     half * 1024:(half + 1) * 1024],
                            in_=st["ostg"][:, half * 1024:(half + 1) * 1024])
                    return f

                for n in range(4):
                    for h in range(HPC):
                        steps.append(chunk_mm(st, tt, n, h))
                    steps.append(chunk_cp(st, tt, n))
                    if n % 2 == 1:
                        steps.append(out_dma(st, tt, n // 2))
            return steps

        # ------- attention for slice j with filler interleave -------

        def emit_attention(j, filler):
            sl = slice(j * 512, (j + 1) * 512)
            ntk = 4 * (j + 1)
            nblocks = HPC * ntk
            bdone = 0

            def pop_filler():
                nonlocal bdone
                bdone += 1
                rem_blocks = nblocks - bdone
                if not filler:
                    return
                if rem_blocks <= 0:
                    while filler:
                        filler.pop(0)()
                    return
                k = (len(filler) + rem_blocks - 1) // rem_blocks
                for _ in range(min(k, len(filler))):
                    filler.pop(0)()

            fin_prev = [None]
            for h in range(HPC):
                st = {"sps": {}, "pt": {}}

                def score(tkb):
                    # causal trim: diagonal tk-block 4j+s only needs
                    # tq_local >= 128*s, so score/exp/AV run on [lo:512]
                    s = tkb - 4 * j
                    lo = 128 * s if s >= 0 else 0
                    sps = psum.tile([128, 512], F32, tag="sp", bufs=3,
                                    name=f"sps{j}_{h}_{tkb}")
                    nc.tensor.matmul(out=sps[:, lo:512],
                                     lhsT=kT[:, tkb * 128:(tkb + 1) * 128],
                                     rhs=qT[h][:, j * 512 + lo:(j + 1) * 512],
                                     start=True, stop=True)
                    pt = work.tile([128, 512], BF16, tag="pt", bufs=6,
                                   name=f"pt{j}_{h}_{tkb}")
                    nc.scalar.activation(out=pt[:, lo:512],
                                         in_=sps[:, lo:512],
                                         func=mybir.ActivationFunctionType.Exp,
                                         scale=SCALE)
                    if s >= 0:
                        # strict upper triangle of the 128x128 tile at the
                        # diagonal: zero where local tq < tk
                        nc.gpsimd.affine_select(
                            out=pt[:, lo:lo + 128], in_=pt[:, lo:lo + 128],
                            compare_op=mybir.AluOpType.is_ge,
                            fill=0.0,
                            base=0,
                            pattern=[[1, 128]],
                            channel_multiplier=-1,
                        )
                    st["pt"][tkb] = (pt, lo)

                def consume(tkb):
                    pt, lo = st["pt"].pop(tkb)
                    if tkb == 0:
                        st["otps"] = psum.tile([128, 512], F32, tag="ot",
                                               bufs=2, name=f"otps{j}_{h}")
                        st["racc"] = work.tile([128, 512], BF16, tag="racc",
                                               bufs=2, name=f"racc{j}_{h}")
                    nc.tensor.matmul(out=st["otps"][:, lo:512],
                                     lhsT=v_sb[tkb], rhs=pt[:, lo:512],
                                     start=(tkb == 0), stop=(tkb == ntk - 1))
                    if tkb == 0:
                        nc.vector.tensor_copy(out=st["racc"], in_=pt)
                    else:
                        nc.vector.tensor_add(out=st["racc"][:, lo:512],
                                             in0=st["racc"][:, lo:512],
                                             in1=pt[:, lo:512])

                def make_fin(h, st):
                    # finalize head: rowsum via ones-matmul, normalize.
                    # Deferred into the NEXT head's stream so the rowsum MM
                    # never heads the PE queue while the DVE racc chain is
                    # still draining.
                    def fin():
                        rsb = psum.tile([128, 512], F32, tag="acc", bufs=3,
                                        name=f"rsb{j}_{h}")
                        nc.tensor.matmul(out=rsb, lhsT=ones_bf,
                                         rhs=st["racc"],
                                         start=True, stop=True)
                        rinv = work.tile([128, 512], F32, tag="rinv", bufs=2,
                                         name=f"rinv{j}_{h}")
                        nc.vector.reciprocal_approx_fast(rinv, rsb)
                        nc.vector.tensor_mul(out=oT[h][:, sl],
                                             in0=st["otps"], in1=rinv)
                    return fin

                # software pipeline: score k+3 runs ahead of AV k (covers
                # the exp + causal-mask latency of diagonal blocks)
                depth = min(3, ntk - 1)
                for tkb in range(ntk):
                    score(tkb)
                    if tkb == 1 and fin_prev[0] is not None:
                        fin_prev[0]()
                        fin_prev[0] = None
                    if tkb >= depth:
                        consume(tkb - depth)
                    pop_filler()
                for tkb in range(max(0, ntk - depth), ntk):
                    consume(tkb)
                fin_prev[0] = make_fin(h, st)

            # last head's FIN goes behind a bit of leftover filler (filler
            # never reads this slice's oT, so this is order-safe)
            for _ in range(min(6, len(filler))):
                filler.pop(0)()
            fin_prev[0]()
            fin_prev[0] = None
            # leftover filler (normally consumed inside the loop)
            while filler:
                filler.pop(0)()

        # ------- program -------
        # prologue: slice-0 projections, ordered to track the DMA stream.
        # K and V interleave per xs0 chunk (their weights land before wq)
        # so the PE has maximal ready work per landed byte; Q heads 0/1 run
        # their wq_a halves first so they never wait on wq_b.
        K0 = gen_proj(0, "k")
        V0 = gen_proj(0, "v")
        Qs = [gen_proj(0, "q", hh) for hh in range(HPC)]
        for f in (K0[0:8] + V0[0:8] + K0[8:12] + V0[8:12]
                  + K0[12:16] + V0[12:16] + [K0[16]] + V0[16:]
                  + Qs[0][0:8] + Qs[1][0:8] + Qs[0][8:17] + Qs[1][8:17]
                  + Qs[2] + Qs[3]):
            f()

        # filler distribution: defer half of oproj(1) to slice 3 so the
        # long slice-3 attention keeps enough PE filler
        for j in range(NSL):
            filler = []
            if j + 1 < NSL:
                filler += gen_proj(j + 1, "k")
                for h in range(HPC):
                    filler += gen_proj(j + 1, "q", h)
                filler += gen_proj(j + 1, "v")
            if j == 1:
                filler += gen_oproj(0)
            elif j == 3:
                filler += gen_oproj(1)
                filler += gen_oproj(2)
            emit_attention(j, filler)

        # epilogue: last slice's output projection
        for f in gen_oproj(NSL - 1):
            f()

    nc.compile()
    return nc


def _get_nc():
    if "nc" not in _CACHE:
        _CACHE["nc"] = _build_nc()
    return _CACHE["nc"]


def _bf16(a):
    return np.ascontiguousarray(a.astype(ml_dtypes.bfloat16))


def kernel(x, Wq, bq, Wk, bk, Wv, bv, Wo, bo, **kw):
    x = np.asarray(x, dtype=np.float32)
    Wq = np.asarray(Wq, dtype=np.float32)
    Wk = np.asarray(Wk, dtype=np.float32)
    Wv = np.asarray(Wv, dtype=np.float32)
    Wo = np.asarray(Wo, dtype=np.float32)
    bq = np.asarray(bq, dtype=np.float32)
    bk = np.asarray(bk, dtype=np.float32)
    bv = np.asarray(bv, dtype=np.float32)
    bo = np.asarray(bo, dtype=np.float32)

    nc = _get_nc()

    # x slices, shared per batch: xs[b][j] = [128, 16*512] with columns
    # (kb, t') st. xs[b][j][p, kb*512+t'] = x[b, j*512+t', kb*128+p]
    xs_b = []
    for b in range(B):
        xT = np.ascontiguousarray(x[b].T)            # [D, T]
        xs = xT.reshape(16, 128, NSL, 512).transpose(2, 1, 0, 3)
        xs_b.append(_bf16(xs.reshape(NSL, 128, 16 * 512)))

    # per head-quarter weight packs, shared across batches
    packs = []
    for q in range(HPC):
        hs = q * HPC * DH
        kv = q // 2
        wqp = _bf16(Wq[:, hs:hs + HPC * DH].reshape(16, 128, HPC * DH)
                    .transpose(1, 0, 2).reshape(128, 16 * 512))
        wkp = _bf16(Wk[:, kv * DH:(kv + 1) * DH].reshape(16, 128, DH)
                    .transpose(1, 0, 2).reshape(128, 16 * 128))
        wvp = _bf16(Wv[:, kv * DH:(kv + 1) * DH].reshape(16, 128, DH)
                    .transpose(1, 0, 2).reshape(128, 16 * 128))
        wop = _bf16(Wo[hs:hs + HPC * DH, :].reshape(HPC, 128, D)
                    .transpose(1, 0, 2).reshape(128, HPC * D))
        bq_m = np.ascontiguousarray(
            bq[hs:hs + HPC * DH].reshape(HPC, DH).T)          # [128, 4]
        bk_m = np.ascontiguousarray(
            bk[kv * DH:(kv + 1) * DH].reshape(DH, 1))         # [128, 1]
        bv_m = np.ascontiguousarray(
            bv[kv * DH:(kv + 1) * DH].reshape(DH, 1))         # [128, 1]
        packs.append((wqp, wkp, wvp, wop, bq_m, bk_m, bv_m))

    in_maps = []
    for c in range(NCORES):
        b = c // 4
        q = c % 4
        wqp, wkp, wvp, wop, bq_m, bk_m, bv_m = packs[q]
        m = {f"xs{j}": xs_b[b][j] for j in range(NSL)}
        m.update({
            "wqx": wqp, "wkx": wkp, "wvx": wvp, "wox": wop,
            "bqm": bq_m, "bkm": bk_m, "bvm": bv_m,
        })
        in_maps.append(m)

    res = run_bass_kernel_spmd(nc, in_maps, list(range(NCORES)),
                               **kw.get("_run_kwargs", {}))
    if kw.get("_return_res"):
        return res
    parts = [res.results[c]["part"] for c in range(NCORES)]
    out = np.empty((B, T, D), dtype=np.float32)
    for b in range(B):
        acc = parts[4 * b].astype(np.float32)
        for q in range(1, 4):
            acc = acc + parts[4 * b + q].astype(np.float32)
        out[b] = acc + bo[None, :]
    return out


# revision 26
# speedup vs baseline: 1.5240x; 1.0019x over previous
"""GQA kernel for Trainium2, 8 NeuronCores.

Problem: B=2, T=2048, D=2048, 16 query heads / 2 KV heads, d_head=128, causal.

Sharding: core c -> batch b = c//4, head-quarter q = c%4 (query heads
4q..4q+3, kv head q//2). Each core computes its 4 heads' attention and a
partial output projection (its Wo rows); host sums the 4 partials per batch
and adds bo.

Host marshalling: all inputs pre-cast to bf16 and packed into [128, N]
arrays whose column layout equals the SBUF tile layout, so each logical
group is one large DMA with multi-KB contiguous rows, issued on a single
HWDGE queue in exact consumption order:
  wk, x-slice0 (4 chunks), wv, wq (2 chunks), x-slice1, wo, x-s2, x-s3.

On-core dataflow (bf16 matmuls, fp32 PSUM accum), 4 rounds over 512-wide
t-slices. Per slice j the ACT-heavy attention blocks (score -> exp ->
[causal mask] -> AV) are emitted software-pipelined (score k+3 ahead of
AV k, covering exp + mask latency) with the PE-dense filler work
(projections of slice j+1, output projection of an earlier slice) spread
between them a few matmuls at a time, so the statically-ordered PE queue
never waits on ACT/DVE progress.  Each head's finalize (rowsum matmul,
reciprocal, normalize) is deferred into the next head's stream so it
never heads the PE queue while the DVE row-sum chain drains.

PSUM: sps bufs=3 (tag sp), otps bufs=2 (tag ot, long-lived across the tk
loop), everything else (proj accums, rowsum, oproj, V-transpose) rotates
through tag acc bufs=3.  Row sums accumulate on DVE in bf16 (2x mode);
output partials are written bf16 (host sums in fp32).
"""

import numpy as np
import ml_dtypes
from contextlib import ExitStack

import concourse.bass as bass
from concourse import bacc
import concourse.mybir as mybir
import concourse.tile as tile
from concourse.bass_utils import run_bass_kernel_spmd
from concourse.masks import make_identity

F32 = mybir.dt.float32
BF16 = mybir.dt.bfloat16

D = 2048
T = 2048
DH = 128
B = 2
HPC = 4            # query heads per core
NCORES = 8
NSL = 4            # t-slices of 512
SCALE = 1.0 / float(np.sqrt(128.0))

_CACHE = {}


def _build_nc():
    nc = bacc.Bacc("TRN2", target_bir_lowering=False, debug=False,
                   num_devices=NCORES)

    # packed inputs: one dram tensor per DMA group, rows are the SBUF
    # partition lines (multi-KB contiguous per row)
    xsd = [nc.dram_tensor(f"xs{j}", [128, 16 * 512], BF16,
                          kind="ExternalInput") for j in range(NSL)]
    wkd = nc.dram_tensor("wkx", [128, 16 * 128], BF16, kind="ExternalInput")
    wqd = nc.dram_tensor("wqx", [128, 16 * 512], BF16, kind="ExternalInput")
    wvd = nc.dram_tensor("wvx", [128, 16 * 128], BF16, kind="ExternalInput")
    wod = nc.dram_tensor("wox", [128, HPC * D], BF16, kind="ExternalInput")
    bqm = nc.dram_tensor("bqm", [DH, HPC], F32, kind="ExternalInput")
    bkm = nc.dram_tensor("bkm", [DH, 1], F32, kind="ExternalInput")
    bvm = nc.dram_tensor("bvm", [DH, 1], F32, kind="ExternalInput")
    part = nc.dram_tensor("part", [T, D], BF16, kind="ExternalOutput")

    with ExitStack() as ctx:
        tc = ctx.enter_context(tile.TileContext(nc))
        persist = ctx.enter_context(tc.tile_pool(name="persist", bufs=1))
        work = ctx.enter_context(tc.tile_pool(name="work", bufs=3))
        psum = ctx.enter_context(tc.tile_pool(name="psum", bufs=2, space="PSUM"))

        # ---- constants ----
        ones_bf = persist.tile([128, 128], BF16, tag="ones", name="ones_bf")
        nc.vector.memset(ones_bf, 1.0)
        ident = persist.tile([128, 128], BF16, tag="ident", name="ident")
        make_identity(nc, ident)

        bq_sb = persist.tile([DH, HPC], F32, tag="bq", name="bq_sb")
        nc.scalar.dma_start(out=bq_sb, in_=bqm[:, :])
        bk_sb = persist.tile([DH, 1], F32, tag="bk", name="bk_sb")
        nc.scalar.dma_start(out=bk_sb, in_=bkm[:, :])
        bv_sb = persist.tile([DH, 1], F32, tag="bv", name="bv_sb")
        nc.scalar.dma_start(out=bv_sb, in_=bvm[:, :])

        # ---- inputs -> SBUF on the sync HWDGE queue, in exact consumption
        # order.  Slice-0 groups are split fine-grained so the first
        # projection matmuls start as early as possible. ----
        xs0_sb = [persist.tile([128, 2 * 512], BF16, tag=f"xs0_{g}",
                               name=f"xs0_sb{g}") for g in range(8)]
        xs_sb = [None] + [persist.tile([128, 16 * 512], BF16, tag=f"xs{j}",
                                       name=f"xs_sb{j}") for j in range(1, NSL)]
        wk_sb = persist.tile([128, 16 * 128], BF16, tag="wk", name="wk_sb")
        wq2_sb = [persist.tile([128, 8 * 512], BF16, tag=f"wq{g}",
                               name=f"wq_sb{g}") for g in range(2)]
        wv_sb = persist.tile([128, 16 * 128], BF16, tag="wv", name="wv_sb")
        wo_sb = persist.tile([128, HPC * D], BF16, tag="wo", name="wo_sb")

        nc.sync.dma_start(out=wk_sb[:, 0:512], in_=wkd[:, 0:512])
        nc.sync.dma_start(out=wk_sb[:, 512:2048], in_=wkd[:, 512:2048])
        for g in range(4):
            nc.sync.dma_start(out=xs0_sb[g],
                              in_=xsd[0][:, g * 1024:(g + 1) * 1024])
        nc.sync.dma_start(out=wv_sb, in_=wvd[:, :])
        for g in range(4, 8):
            nc.sync.dma_start(out=xs0_sb[g],
                              in_=xsd[0][:, g * 1024:(g + 1) * 1024])
        for g in range(2):
            nc.sync.dma_start(out=wq2_sb[g],
                              in_=wqd[:, g * 4096:(g + 1) * 4096])
        nc.sync.dma_start(out=xs_sb[1], in_=xsd[1][:, :])
        nc.sync.dma_start(out=wo_sb, in_=wod[:, :])
        nc.sync.dma_start(out=xs_sb[2], in_=xsd[2][:, :])
        nc.sync.dma_start(out=xs_sb[3], in_=xsd[3][:, :])

        def xrhs(j, kb):
            if j == 0:
                return xs0_sb[kb // 2][:, (kb % 2) * 512:(kb % 2 + 1) * 512]
            return xs_sb[j][:, kb * 512:(kb + 1) * 512]

        def wqap(kb, h):
            return wq2_sb[kb // 8][:, (kb % 8) * 512 + h * 128:
                                   (kb % 8) * 512 + (h + 1) * 128]

        # ---- persistent activations ----
        qT = [persist.tile([128, T], BF16, tag=f"qT{h}", name=f"qT{h}")
              for h in range(HPC)]
        kT = persist.tile([128, T], BF16, tag="kT", name="kT")
        v_sb = [persist.tile([128, DH], BF16, tag=f"v{t}", name=f"v{t}")
                for t in range(16)]
        oT = [persist.tile([128, T], BF16, tag=f"oT{h}", name=f"oT{h}")
              for h in range(HPC)]

        # ------- emission helpers; each returns a list of closures that
        # emit ONE instruction each (plus tile allocs), used as PE filler
        # interleaved into the attention stream -------

        def gen_proj(j, kind, h=0):
            """Projection group for slice j: 16 accumulating MMs + ACT
            epilogue. kind in {'k','q','v'}."""
            sl = slice(j * 512, (j + 1) * 512)
            st = {}

            def mm(kb):
                def f():
                    if kb == 0:
                        st["ps"] = psum.tile([128, 512], F32, tag="acc",
                                             bufs=3, name=f"{kind}ps{j}_{h}")
                    if kind == "k":
                        lhsT = wk_sb[:, kb * 128:(kb + 1) * 128]
                    elif kind == "v":
                        lhsT = wv_sb[:, kb * 128:(kb + 1) * 128]
                    else:
                        lhsT = wqap(kb, h)
                    nc.tensor.matmul(out=st["ps"], lhsT=lhsT,
                                     rhs=xrhs(j, kb),
                                     start=(kb == 0), stop=(kb == 15))
                return f

            steps = [mm(kb) for kb in range(16)]

            if kind == "k":
                def epi():
                    nc.scalar.activation(out=kT[:, sl], in_=st["ps"],
                                         func=mybir.ActivationFunctionType.Identity,
                                         bias=bk_sb[:, 0:1], scale=1.0)
                steps.append(epi)
            elif kind == "q":
                def epi():
                    nc.scalar.activation(out=qT[h][:, sl], in_=st["ps"],
                                         func=mybir.ActivationFunctionType.Identity,
                                         bias=bq_sb[:, h:h + 1], scale=1.0)
                steps.append(epi)
            else:
                def epi():
                    st["vt"] = work.tile([128, 512], BF16, tag="vt", bufs=2,
                                         name=f"vt{j}")
                    nc.scalar.activation(out=st["vt"], in_=st["ps"],
                                         func=mybir.ActivationFunctionType.Identity,
                                         bias=bv_sb[:, 0:1], scale=1.0)
                steps.append(epi)

                def tr(sub):
                    def f():
                        if sub == 0:
                            st["vtp"] = psum.tile([128, 512], BF16, tag="acc",
                                                  bufs=3, name=f"vtp{j}")
                        nc.tensor.transpose(
                            st["vtp"][:, sub * 128:(sub + 1) * 128],
                            st["vt"][:, sub * 128:(sub + 1) * 128], ident)
                    return f

                def cp(sub):
                    def f():
                        nc.vector.tensor_copy(
                            out=v_sb[4 * j + sub],
                            in_=st["vtp"][:, sub * 128:(sub + 1) * 128])
                    return f

                for sub in range(4):
                    steps.append(tr(sub))
                steps += [cp(sub) for sub in range(4)]
            return steps

        def gen_oproj(j, tiles=range(4)):
            """Output projection for t-tiles of slice j.  Each 512-chunk:
            4 accumulating MMs + DVE copy to bf16 staging; chunked output
            DMAs on the scalar HWDGE queue."""
            steps = []
            for tt in [4 * j + s for s in tiles]:
                st = {}

                def chunk_mm(st, tt, n, h):
                    def f():
                        if n == 0 and h == 0:
                            st["ostg"] = work.tile([128, D], BF16, tag="ostg",
                                                   bufs=2, name=f"ostg{tt}")
                        if h == 0:
                            st["ops"] = psum.tile([128, 512], F32, tag="acc",
                                                  bufs=3, name=f"ops{tt}_{n}")
                        nc.tensor.matmul(
                            out=st["ops"],
                            lhsT=oT[h][:, tt * 128:(tt + 1) * 128],
                            rhs=wo_sb[:, h * D + n * 512:h * D + (n + 1) * 512],
                            start=(h == 0), stop=(h == HPC - 1))
                    return f

                def chunk_cp(st, tt, n):
                    # alternate DVE/ACT so the oproj PSUM release never
                    # queues behind the DVE racc chains
                    def f():
                        if n % 2 == 0:
                            nc.vector.tensor_copy(
                                out=st["ostg"][:, n * 512:(n + 1) * 512],
                                in_=st["ops"])
                        else:
                            nc.scalar.copy(
                                out=st["ostg"][:, n * 512:(n + 1) * 512],
                                in_=st["ops"])
                    return f

                def out_dma(st, tt, half):
                    def f():
                        nc.scalar.dma_start(
                            out=part[tt * 128:(tt + 1) * 128,
                                     half * 1024:(half + 1) * 1024],
                            in_=st["ostg"][:, half * 1024:(half + 1) * 1024])
                    return f

                for n in range(4):
                    for h in range(HPC):
                        steps.append(chunk_mm(st, tt, n, h))
                    steps.append(chunk_cp(st, tt, n))
                    if n % 2 == 1:
                        steps.append(out_dma(st, tt, n // 2))
            return steps

        # ------- attention for slice j with filler interleave -------

        def emit_attention(j, filler):
            sl = slice(j * 512, (j + 1) * 512)
            ntk = 4 * (j + 1)
            nblocks = HPC * (ntk + 4)
            bdone = 0

            def pop_filler():
                nonlocal bdone
                bdone += 1
                rem_blocks = nblocks - bdone
                if not filler:
                    return
                if rem_blocks <= 0:
                    while filler:
                        filler.pop(0)()
                    return
                k = (len(filler) + rem_blocks - 1) // rem_blocks
                for _ in range(min(k, len(filler))):
                    filler.pop(0)()

            fin_prev = [None]
            for h in range(HPC):
                st = {"sps": {}, "pt": {}}

                def score(tkb):
                    # causal trim: diagonal tk-block 4j+s only needs
                    # tq_local >= 128*s, so score/exp/AV run on [lo:512]
                    s = tkb - 4 * j
                    lo = 128 * s if s >= 0 else 0
                    sps = psum.tile([128, 512], F32, tag="sp", bufs=3,
                                    name=f"sps{j}_{h}_{tkb}")
                    nc.tensor.matmul(out=sps[:, lo:512],
                                     lhsT=kT[:, tkb * 128:(tkb + 1) * 128],
                                     rhs=qT[h][:, j * 512 + lo:(j + 1) * 512],
                                     start=True, stop=True)
                    pt = work.tile([128, 512], BF16, tag="pt", bufs=6,
                                   name=f"pt{j}_{h}_{tkb}")
                    nc.scalar.activation(out=pt[:, lo:512],
                                         in_=sps[:, lo:512],
                                         func=mybir.ActivationFunctionType.Exp,
                                         scale=SCALE)
                    if s >= 0:
                        # strict upper triangle of the 128x128 tile at the
                        # diagonal: zero where local tq < tk
                        nc.gpsimd.affine_select(
                            out=pt[:, lo:lo + 128], in_=pt[:, lo:lo + 128],
                            compare_op=mybir.AluOpType.is_ge,
                            fill=0.0,
                            base=0,
                            pattern=[[1, 128]],
                            channel_multiplier=-1,
                        )
                    st["pt"][tkb] = (pt, lo)

                def consume(tkb):
                    pt, lo = st["pt"].pop(tkb)
                    if tkb == 0:
                        st["otps"] = psum.tile([128, 512], F32, tag="ot",
                                               bufs=2, name=f"otps{j}_{h}")
                        st["racc"] = work.tile([128, 512], BF16, tag="racc",
                                               bufs=2, name=f"racc{j}_{h}")
                    nc.tensor.matmul(out=st["otps"][:, lo:512],
                                     lhsT=v_sb[tkb], rhs=pt[:, lo:512],
                                     start=(tkb == 0), stop=(tkb == ntk - 1))
                    if tkb == 0:
                        nc.vector.tensor_copy(out=st["racc"], in_=pt)
                    else:
                        nc.vector.tensor_add(out=st["racc"][:, lo:512],
                                             in0=st["racc"][:, lo:512],
                                             in1=pt[:, lo:512])

                def make_fin(h, st):
                    # finalize head: rowsum via ones-matmul, normalize.
                    # Deferred into the NEXT head's stream so the rowsum MM
                    # never heads the PE queue while the DVE racc chain is
                    # still draining.
                    def fin():
                        rsb = psum.tile([128, 512], F32, tag="acc", bufs=3,
                                        name=f"rsb{j}_{h}")
                        nc.tensor.matmul(out=rsb, lhsT=ones_bf,
                                         rhs=st["racc"],
                                         start=True, stop=True)
                        rinv = work.tile([128, 512], F32, tag="rinv", bufs=2,
                                         name=f"rinv{j}_{h}")
                        nc.vector.reciprocal_approx_fast(rinv, rsb)
                        nc.vector.tensor_mul(out=oT[h][:, sl],
                                             in0=st["otps"], in1=rinv)
                    return fin

                # software pipeline: score k+3 runs ahead of AV k (covers
                # the exp + causal-mask latency of diagonal blocks)
                depth = min(3, ntk - 1)
                for tkb in range(ntk):
                    score(tkb)
                    if tkb == 1 and fin_prev[0] is not None:
                        fin_prev[0]()
                        fin_prev[0] = None
                    if tkb >= depth:
                        consume(tkb - depth)
                    pop_filler()
                    if tkb >= 4 * j:
                        # diagonal blocks have trimmed (shorter) PE work but
                        # full exp+mask chain latency; give them extra filler
                        pop_filler()
                for tkb in range(max(0, ntk - depth), ntk):
                    consume(tkb)
                fin_prev[0] = make_fin(h, st)

            # last head's FIN goes behind a bit of leftover filler (filler
            # never reads this slice's oT, so this is order-safe)
            for _ in range(min(6, len(filler))):
                filler.pop(0)()
            fin_prev[0]()
            fin_prev[0] = None
            # leftover filler (normally consumed inside the loop)
            while filler:
                filler.pop(0)()

        # ------- program -------
        # prologue: slice-0 projections, ordered to track the DMA stream.
        # K and V interleave per xs0 chunk (their weights land before wq)
        # so the PE has maximal ready work per landed byte; Q heads 0/1 run
        # their wq_a halves first so they never wait on wq_b.
        K0 = gen_proj(0, "k")
        V0 = gen_proj(0, "v")
        Qs = [gen_proj(0, "q", hh) for hh in range(HPC)]
        # V transposes (V0[17:21]) go behind Q-head-0's first matmuls so the
        # PE never waits on the V epilogue (ACT) right at the V/Q boundary
        for f in (K0[0:8] + V0[0:8] + K0[8:12] + V0[8:12]
                  + K0[12:16] + V0[12:16] + [K0[16], V0[16]]
                  + Qs[0][0:8] + V0[17:] + Qs[1][0:8]
                  + Qs[0][8:17] + Qs[1][8:17] + Qs[2] + Qs[3]):
            f()

        # filler distribution: defer half of oproj(1) to slice 3 so the
        # long slice-3 attention keeps enough PE filler
        for j in range(NSL):
            filler = []
            if j + 1 < NSL:
                filler += gen_proj(j + 1, "k")
                for h in range(HPC):
                    filler += gen_proj(j + 1, "q", h)
                filler += gen_proj(j + 1, "v")
            if j == 1:
                filler += gen_oproj(0)
            elif j == 3:
                filler += gen_oproj(1)
                filler += gen_oproj(2)
            emit_attention(j, filler)

        # epilogue: last slice's output projection
        for f in gen_oproj(NSL - 1):
            f()

    nc.compile()
    return nc


def _get_nc():
    if "nc" not in _CACHE:
        _CACHE["nc"] = _build_nc()
    return _CACHE["nc"]


def _bf16(a):
    return np.ascontiguousarray(a.astype(ml_dtypes.bfloat16))


def kernel(x, Wq, bq, Wk, bk, Wv, bv, Wo, bo, **kw):
    x = np.asarray(x, dtype=np.float32)
    Wq = np.asarray(Wq, dtype=np.float32)
    Wk = np.asarray(Wk, dtype=np.float32)
    Wv = np.asarray(Wv, dtype=np.float32)
    Wo = np.asarray(Wo, dtype=np.float32)
    bq = np.asarray(bq, dtype=np.float32)
    bk = np.asarray(bk, dtype=np.float32)
    bv = np.asarray(bv, dtype=np.float32)
    bo = np.asarray(bo, dtype=np.float32)

    nc = _get_nc()

    # x slices, shared per batch: xs[b][j] = [128, 16*512] with columns
    # (kb, t') st. xs[b][j][p, kb*512+t'] = x[b, j*512+t', kb*128+p]
    xs_b = []
    for b in range(B):
        xT = np.ascontiguousarray(x[b].T)            # [D, T]
        xs = xT.reshape(16, 128, NSL, 512).transpose(2, 1, 0, 3)
        xs_b.append(_bf16(xs.reshape(NSL, 128, 16 * 512)))

    # per head-quarter weight packs, shared across batches
    packs = []
    for q in range(HPC):
        hs = q * HPC * DH
        kv = q // 2
        wqp = _bf16(Wq[:, hs:hs + HPC * DH].reshape(16, 128, HPC * DH)
                    .transpose(1, 0, 2).reshape(128, 16 * 512))
        wkp = _bf16(Wk[:, kv * DH:(kv + 1) * DH].reshape(16, 128, DH)
                    .transpose(1, 0, 2).reshape(128, 16 * 128))
        wvp = _bf16(Wv[:, kv * DH:(kv + 1) * DH].reshape(16, 128, DH)
                    .transpose(1, 0, 2).reshape(128, 16 * 128))
        wop = _bf16(Wo[hs:hs + HPC * DH, :].reshape(HPC, 128, D)
                    .transpose(1, 0, 2).reshape(128, HPC * D))
        bq_m = np.ascontiguousarray(
            bq[hs:hs + HPC * DH].reshape(HPC, DH).T)          # [128, 4]
        bk_m = np.ascontiguousarray(
            bk[kv * DH:(kv + 1) * DH].reshape(DH, 1))         # [128, 1]
        bv_m = np.ascontiguousarray(
            bv[kv * DH:(kv + 1) * DH].reshape(DH, 1))         # [128, 1]
        packs.append((wqp, wkp, wvp, wop, bq_m, bk_m, bv_m))

    in_maps = []
    for c in range(NCORES):
        b = c // 4
        q = c % 4
        wqp, wkp, wvp, wop, bq_m, bk_m, bv_m = packs[q]
        m = {f"xs{j}": xs_b[b][j] for j in range(NSL)}
        m.update({
            "wqx": wqp, "wkx": wkp, "wvx": wvp, "wox": wop,
            "bqm": bq_m, "bkm": bk_m, "bvm": bv_m,
        })
        in_maps.append(m)

    res = run_bass_kernel_spmd(nc, in_maps, list(range(NCORES)),
                               **kw.get("_run_kwargs", {}))
    if kw.get("_return_res"):
        return res
    parts = [res.results[c]["part"] for c in range(NCORES)]
    out = np.empty((B, T, D), dtype=np.float32)
    for b in range(B):
        acc = parts[4 * b].astype(np.float32)
        for q in range(1, 4):
            acc = acc + parts[4 * b + q].astype(np.float32)
        out[b] = acc + bo[None, :]
    return out
